# revision 1
# baseline (speedup 1.0000x reference)
"""Multi-head attention (B=2, S=2048, H=16, D=64) on 8 TRN2 NeuronCores.

Sharding: data parallel on batch (2) x tensor parallel on heads (16 -> 4 per
core).  Core c handles batch c//4 and heads [4*(c%4), 4*(c%4)+4).  Each core
projects q/k/v for its head group from its batch's activations, runs the
full S x S attention for its 4 heads, and writes ctx in [head, D, S] layout.
The host transposes/concatenates shards (not part of HW exec time).

Device kernel (per core, identical SPMD program, no collectives):
  - qT/kT computed directly in [D, S] layout (head pairs packed into 128
    partitions) so the scores matmul needs no transposes.
  - scoresT tiles [S_k=128, S_q] = kT_chunk.T @ qT; softmax denominator via a
    ones-column appended to v (one matmul stream produces ctx and denom).
  - exp on the scalar engine with the 1/sqrt(D) scale folded in; bf16
    matmul operands, f32 PSUM accumulation; final normalize = DVE divide.
  - padding mask folded into v_aug row zeroing (exp(x-1e4) underflows to 0
    in f32, so zeroing masked key rows is exactly equivalent).
"""

import numpy as np
import ml_dtypes

import concourse.bass as bass
import concourse.tile as tile
from concourse import bacc, mybir
from concourse.bass_utils import run_bass_kernel_spmd

B, S, H, D = 2, 2048, 16, 64
HID = H * D
NCORES = 8
HPC = 4               # heads per core
COLS = HPC * D        # 256 projection columns per core
KC = HID // 128       # 8 contraction chunks for projections
QC = S // 512         # 4 query chunks of 512
MC = S // 128         # 16 key chunks of 128

BF16 = mybir.dt.bfloat16
F32 = mybir.dt.float32
np_bf16 = ml_dtypes.bfloat16

_CACHE = {}


def build(apply_mask: bool) -> bass.Bass:
    nc = bacc.Bacc(None, target_bir_lowering=False, debug=False)

    xT = nc.declare_dram_parameter("xT", [HID, S], BF16, isOutput=False)
    wq = nc.declare_dram_parameter("wq", [HID, COLS], BF16, isOutput=False)
    wk = nc.declare_dram_parameter("wk", [HID, COLS], BF16, isOutput=False)
    wv = nc.declare_dram_parameter("wv", [HID, COLS], BF16, isOutput=False)
    bq = nc.declare_dram_parameter("bq", [128, 2], F32, isOutput=False)
    bk = nc.declare_dram_parameter("bk", [128, 2], F32, isOutput=False)
    bv = nc.declare_dram_parameter("bv", [128, COLS], F32, isOutput=False)
    if apply_mask:
        mm_in = nc.declare_dram_parameter("maskm", [128, MC], F32, isOutput=False)
    out_ext = nc.declare_dram_parameter("out", [HPC, D, S], F32, isOutput=True)

    with tile.TileContext(nc) as tc:
        with (
            tc.tile_pool(name="singles", bufs=1) as singles,
            tc.tile_pool(name="work", bufs=4) as work,
            tc.tile_pool(name="psum", bufs=2, space="PSUM") as psum,
        ):
            # ---- input DMA ----
            # x split 16 ways so every DMA queue pulls a slice concurrently;
            # weights queued after x so they don't steal bandwidth from the
            # tensor everything gates on
            # tiny biases first (they gate the projection epilogues), then wk,
            # then the big x tensor 16-way split, then wv/wq in consumption
            # order — so nothing small lands behind x in a queue
            bq_sb = singles.tile([128, 2], F32)
            nc.sync.dma_start(out=bq_sb, in_=bq[:, :])
            bk_sb = singles.tile([128, 2], F32)
            nc.sync.dma_start(out=bk_sb, in_=bk[:, :])
            bv_sb = singles.tile([128, COLS], F32)
            nc.sync.dma_start(out=bv_sb, in_=bv[:, :])
            if apply_mask:
                mm_sb = singles.tile([128, MC], F32)
                nc.sync.dma_start(out=mm_sb, in_=mm_in[:, :])

            wq_sb = singles.tile([128, KC, COLS], BF16)
            wk_sb = singles.tile([128, KC, COLS], BF16)
            wv_sb = singles.tile([128, KC, COLS], BF16)
            x_sb = singles.tile([128, KC, S], BF16)
            for kc in range(KC):
                for h2 in range(2):
                    csl = slice(h2 * (S // 2), (h2 + 1) * (S // 2))
                    nc.sync.dma_start(out=x_sb[:, kc, csl],
                                      in_=xT[kc * 128:(kc + 1) * 128, csl])
                nc.sync.dma_start(out=wk_sb[:, kc, :], in_=wk[kc * 128:(kc + 1) * 128, :])
                nc.sync.dma_start(out=wq_sb[:, kc, :], in_=wq[kc * 128:(kc + 1) * 128, :])

            for kc in range(KC):
                nc.sync.dma_start(out=wv_sb[:, kc, :], in_=wv[kc * 128:(kc + 1) * 128, :])

            # HAM warm-up: the DMA-staggered projection matmuls are too sparse
            # to trip the PE activity window, leaving the whole projection +
            # early-attention phase at the cold 1.2GHz clock. Burn ~4us of
            # dummy matmuls on the first-arriving weight chunk so the array is
            # warm when real work starts; output is never read.
            warm_ps = psum.tile([128, COLS], F32, tag="proj_ps", name="warm_ps")
            for i in range(20):
                nc.tensor.matmul(warm_ps, lhsT=wk_sb[:, 0, 0:128],
                                 rhs=wk_sb[:, 0, :],
                                 start=(i == 0), stop=(i == 19))

            # ---- projections ----
            # kT: [128, pair, S]; partitions 0:64 = head 2p, 64:128 = head 2p+1
            # qTz: zero-padded per head so score matmuls run full-row K=128
            # (variant 0: head-a rows live, b rows zero; variant 1 reversed)
            qTz = singles.tile([128, 2, 2, S], BF16)
            kT = singles.tile([128, 2, S], BF16)
            _zeroed = set()

            def zero_qTz(p, qc):
                if (p, qc) in _zeroed:
                    return
                _zeroed.add((p, qc))
                qsl = slice(qc * 512, (qc + 1) * 512)
                nc.vector.memset(qTz[64:128, p, 0, qsl], 0.0)
                nc.vector.memset(qTz[0:64, p, 1, qsl], 0.0)

            zero_qTz(0, 0)
            # v_aug: [128, key_chunk, head, 128]; cols 64:128 are ones columns,
            # so the ctx matmul emits the softmax denominator replicated into
            # psum partitions 64:128 at no extra cost (matmul cost is N-bound)
            v_aug = singles.tile([128, MC, HPC, 128], BF16)
            nc.vector.memset(v_aug[:, :, :, 64:128], 1.0)

            def project_T_qc(dst, w_sb, b_sb, p, qc, zpad=False):
                ps = psum.tile([128, 512], F32, tag="proj_ps", name=f"pt_{nc.next_id()}")
                for kc in range(KC):
                    nc.tensor.matmul(
                        ps,
                        lhsT=w_sb[:, kc, p * 128:(p + 1) * 128],
                        rhs=x_sb[:, kc, qc * 512:(qc + 1) * 512],
                        start=(kc == 0), stop=(kc == KC - 1),
                    )
                qsl = slice(qc * 512, (qc + 1) * 512)
                if zpad:
                    zero_qTz(p, qc)
                    nc.vector.tensor_tensor(
                        out=dst[0:64, p, 0, qsl],
                        in0=ps[0:64, :],
                        in1=b_sb[0:64, p:p + 1].to_broadcast([64, 512]),
                        op=mybir.AluOpType.add,
                    )
                    nc.vector.tensor_tensor(
                        out=dst[64:128, p, 1, qsl],
                        in0=ps[64:128, :],
                        in1=b_sb[64:128, p:p + 1].to_broadcast([64, 512]),
                        op=mybir.AluOpType.add,
                    )
                else:
                    nc.vector.tensor_tensor(
                        out=dst[:, p, qsl],
                        in0=ps,
                        in1=b_sb[:, p:p + 1].to_broadcast([128, 512]),
                        op=mybir.AluOpType.add,
                    )

            def project_T(dst, w_sb, b_sb, p, zpad=False):
                for qc in range(QC):
                    project_T_qc(dst, w_sb, b_sb, p, qc, zpad)

            def project_v_chunk(mc):
                ps = psum.tile([128, COLS], F32, tag="proj_ps", name=f"pv_{nc.next_id()}")
                for kc in range(KC):
                    nc.tensor.matmul(
                        ps,
                        lhsT=x_sb[:, kc, mc * 128:(mc + 1) * 128],
                        rhs=wv_sb[:, kc, :],
                        start=(kc == 0), stop=(kc == KC - 1),
                    )
                nc.vector.tensor_tensor(
                    out=v_aug[:, mc, :, 0:64],
                    in0=ps[:, :].rearrange("p (h d) -> p h d", h=HPC),
                    in1=bv_sb.rearrange("p (h d) -> p h d", h=HPC),
                    op=mybir.AluOpType.add,
                )
                if apply_mask:
                    nc.vector.tensor_tensor(
                        out=v_aug[:, mc, :, :],
                        in0=v_aug[:, mc, :, :],
                        in1=mm_sb[:, mc:mc + 1, None].to_broadcast([128, HPC, 128]),
                        op=mybir.AluOpType.mult,
                    )

            def attention(p, emit_v=False, pre_qc=None, mid_kc2=None, post_qc=None):
                ha, hb = 2 * p, 2 * p + 1
                for qc in range(QC):
                    if pre_qc is not None:
                        pre_qc(qc)
                    qsl = slice(qc * 512, (qc + 1) * 512)
                    ctx_a = psum.tile([128, 512], F32, tag="ctx", name=f"ca_{nc.next_id()}")
                    ctx_b = psum.tile([128, 512], F32, tag="ctx", name=f"cb_{nc.next_id()}")
                    for kc2 in range(MC // 2):
                        kc0, kc1 = 2 * kc2, 2 * kc2 + 1
                        if mid_kc2 is not None:
                            mid_kc2(qc, kc2)
                        if emit_v and qc == 0:
                            project_v_chunk(kc0)
                            project_v_chunk(kc1)
                        s_a = psum.tile([128, 1024], F32, tag="sps", name=f"sa_{nc.next_id()}")
                        s_b = psum.tile([128, 1024], F32, tag="sps", name=f"sb_{nc.next_id()}")
                        # paired row-group matmuls: head a on array rows 0:63,
                        # head b on rows 64:127 run concurrently
                        for i, kc in enumerate((kc0, kc1)):
                            ksl = slice(kc * 128, (kc + 1) * 128)
                            csl = slice(i * 512, (i + 1) * 512)
                            nc.tensor.matmul(
                                s_a[:, csl], lhsT=kT[:, p, ksl], rhs=qTz[:, p, 0, qsl],
                                start=True, stop=True)
                            nc.tensor.matmul(
                                s_b[:, csl], lhsT=kT[:, p, ksl], rhs=qTz[:, p, 1, qsl],
                                start=True, stop=True)
                        e_a = work.tile([128, 1024], BF16, tag="expT", name=f"ea_{nc.next_id()}")
                        e_b = work.tile([128, 1024], BF16, tag="expT", name=f"eb_{nc.next_id()}")
                        nc.scalar.activation(e_a, s_a, mybir.ActivationFunctionType.Exp,
                                             scale=0.125)
                        nc.scalar.activation(e_b, s_b, mybir.ActivationFunctionType.Exp,
                                             scale=0.125)
                        for i, kc in enumerate((kc0, kc1)):
                            csl = slice(i * 512, (i + 1) * 512)
                            nc.tensor.matmul(
                                ctx_a, lhsT=v_aug[:, kc, ha, :], rhs=e_a[:, csl],
                                start=(kc == 0), stop=(kc == MC - 1))
                            nc.tensor.matmul(
                                ctx_b, lhsT=v_aug[:, kc, hb, :], rhs=e_b[:, csl],
                                start=(kc == 0), stop=(kc == MC - 1))
                    for h, ctx in ((ha, ctx_a), (hb, ctx_b)):
                        # one copy releases the ctx psum bank immediately;
                        # custom-DVE recip needs a base-0 input tile
                        g_sb = work.tile([128, 512], F32, tag="gctx", name=f"g_{nc.next_id()}")
                        nc.vector.tensor_copy(out=g_sb, in_=ctx)
                        d0_sb = work.tile([64, 512], F32, tag="den0", name=f"d0_{nc.next_id()}")
                        nc.vector.tensor_copy(out=d0_sb, in_=g_sb[64:128, :])
                        d_sb = work.tile([64, 512], F32, tag="den", name=f"d_{nc.next_id()}")
                        nc.vector.reciprocal_approx_fast(out=d_sb, in_=d0_sb)
                        o_sb = work.tile([64, 512], F32, tag="outt", name=f"o_{nc.next_id()}")
                        nc.vector.tensor_tensor(
                            out=o_sb, in0=g_sb[0:64, :],
                            in1=d_sb,
                            op=mybir.AluOpType.mult)
                        nc.sync.dma_start(out=out_ext[h][:, qsl], in_=o_sb)
                    if post_qc is not None:
                        post_qc(qc)

            # emission order chosen so attention on pair 0 can start as early
            # as possible; v / pair-1 projections fill PE gaps while the
            # scalar engine grinds pair-0 exps
            # Fine-grained emission: only the first q-chunk of kT0/qT0 precedes
            # attention(0); remaining kT0 chunks are emitted just before the
            # kc2 block that first consumes them, qT0 chunks at each qc start,
            # and pair-1 projections are spread over attention(0)'s qc
            # boundaries so the scalar engine never starves between pairs.
            project_T_qc(kT, wk_sb, bk_sb, 0, 0)
            project_T_qc(qTz, wq_sb, bq_sb, 0, 0, zpad=True)

            # (qc, kc2) -> projection group to emit there: qc0 carries the
            # rest of kT0 plus v (via emit_v); qT0 chunks prefetch one qc
            # ahead; pair-1 groups spread over qc1..3 and attention(1) qc0
            p0_sched = {
                (0, 2): (kT, wk_sb, bk_sb, 0, 1, False),
                (0, 4): (kT, wk_sb, bk_sb, 0, 2, False),
                (0, 6): (kT, wk_sb, bk_sb, 0, 3, False),
                (0, 5): (qTz, wq_sb, bq_sb, 0, 1, True),
                (1, 5): (qTz, wq_sb, bq_sb, 0, 2, True),
                (2, 5): (qTz, wq_sb, bq_sb, 0, 3, True),
                (1, 2): (kT, wk_sb, bk_sb, 1, 0, False),
                (1, 6): (kT, wk_sb, bk_sb, 1, 1, False),
                (2, 2): (kT, wk_sb, bk_sb, 1, 2, False),
                (2, 6): (kT, wk_sb, bk_sb, 1, 3, False),
                (3, 2): (qTz, wq_sb, bq_sb, 1, 0, True),
                (3, 6): (qTz, wq_sb, bq_sb, 1, 1, True),
            }
            p1_sched = {
                (0, 2): (qTz, wq_sb, bq_sb, 1, 2, True),
                (0, 6): (qTz, wq_sb, bq_sb, 1, 3, True),
            }

            def mk_mid(sched):
                def mid(qc, kc2):
                    g = sched.get((qc, kc2))
                    if g is not None:
                        dst, w_sb, b_sb, p, j, zp = g
                        project_T_qc(dst, w_sb, b_sb, p, j, zpad=zp)
                return mid

            attention(0, emit_v=True, mid_kc2=mk_mid(p0_sched))
            attention(1, mid_kc2=mk_mid(p1_sched))

    nc.compile()
    return nc


def _get_nc(apply_mask: bool) -> bass.Bass:
    if apply_mask not in _CACHE:
        _CACHE[apply_mask] = build(apply_mask)
    return _CACHE[apply_mask]


def _in_maps(x, mask, Wq, bq, Wk, bk, Wv, bv, apply_mask):
    xT_b = [np.ascontiguousarray(x[b].T).astype(np_bf16) for b in range(B)]
    maps = []
    for c in range(NCORES):
        b, hg = c // 4, c % 4
        cs = slice(hg * COLS, (hg + 1) * COLS)
        m = {
            "xT": xT_b[b],
            "wq": np.ascontiguousarray(Wq[:, cs]).astype(np_bf16),
            "wk": np.ascontiguousarray(Wk[:, cs]).astype(np_bf16),
            "wv": np.ascontiguousarray(Wv[:, cs]).astype(np_bf16),
            "bq": np.ascontiguousarray(bq[cs].reshape(2, 128).T).astype(np.float32),
            "bk": np.ascontiguousarray(bk[cs].reshape(2, 128).T).astype(np.float32),
            "bv": np.ascontiguousarray(
                np.broadcast_to(bv[cs], (128, COLS))).astype(np.float32),
        }
        if apply_mask:
            m["maskm"] = np.ascontiguousarray(
                mask[b].astype(np.float32).reshape(MC, 128).T)
        maps.append(m)
    return maps


def _ensure_ntff_hook():
    """The agent image's antenv lacks axon_hooks; synthesize it so
    run_bass_kernel_spmd(trace=True) can reach the axon NTFF profiler."""
    import sys as _sys
    import types as _types
    try:
        from antenv import axon_hooks  # noqa: F401
        return
    except ImportError:
        pass
    import antenv
    mod = _types.ModuleType("antenv.axon_hooks")
    _hook = [None]
    mod.set_axon_ntff_profile_hook = lambda h: _hook.__setitem__(0, h)
    mod.get_axon_ntff_profile_hook = lambda: _hook[0]
    _sys.modules["antenv.axon_hooks"] = mod
    antenv.axon_hooks = mod
    from trn_agent_boot.trn_boot import _ntff_profile_via_ctypes
    mod.set_axon_ntff_profile_hook(
        _ntff_profile_via_ctypes("/opt/axon/libaxon_pjrt.so"))


def run(inputs: dict, trace: bool = False):
    if trace:
        _ensure_ntff_hook()
    x = np.asarray(inputs["x"], dtype=np.float32)
    mask = np.asarray(inputs["mask"])
    apply_mask = not bool((mask == 1).all())
    nc = _get_nc(apply_mask)
    maps = _in_maps(x, mask, np.asarray(inputs["Wq"], np.float32),
                    np.asarray(inputs["bq"], np.float32),
                    np.asarray(inputs["Wk"], np.float32),
                    np.asarray(inputs["bk"], np.float32),
                    np.asarray(inputs["Wv"], np.float32),
                    np.asarray(inputs["bv"], np.float32), apply_mask)
    res = run_bass_kernel_spmd(nc, maps, core_ids=list(range(NCORES)), trace=trace)
    out = np.empty((B, S, HID), dtype=np.float32)
    for c in range(NCORES):
        b, hg = c // 4, c % 4
        cs = slice(hg * COLS, (hg + 1) * COLS)
        ctxT = res.results[c]["out"]          # [HPC, D, S]
        out[b, :, cs] = ctxT.transpose(2, 0, 1).reshape(S, COLS)
    return out, res


def kernel(**inputs) -> np.ndarray:
    out, _ = run(inputs)
    return out



# revision 2
# speedup vs baseline: 1.0687x; 1.0687x over previous
"""Multi-head attention (B=2, S=2048, H=16, D=64) on 8 TRN2 NeuronCores.

Sharding: data parallel on batch (2) x tensor parallel on heads (16 -> 4 per
core).  Core c handles batch c//4 and heads [4*(c%4), 4*(c%4)+4).  Each core
projects q/k/v for its head group from its batch's activations, runs the
full S x S attention for its 4 heads, and writes ctx in [head, D, S] layout.
The host transposes/concatenates shards (not part of HW exec time).

Device kernel (per core, identical SPMD program, no collectives):
  - qT/kT computed directly in [D, S] layout (head pairs packed into 128
    partitions) so the scores matmul needs no transposes.
  - scoresT tiles [S_k=128, S_q] = kT_chunk.T @ qTz; softmax denominator via
    ones columns appended to v (one matmul stream produces ctx and denom).
  - exp on the scalar engine with the 1/sqrt(D) scale folded in; bf16
    matmul operands, f32 PSUM accumulation; final normalize = DVE divide.
  - padding mask folded into v_aug row zeroing (exp(x-1e4) underflows to 0
    in f32, so zeroing masked key rows is exactly equivalent).

v2 scheduling (vs the first working version):
  - 8 blocks B(p,qc); block 1 carries only its mandatory projections
    (kT0 all keys, qT0 qc0, all v chunks, plus Q01 prefetch); the
    remaining 10 projection groups are spread one or two per block so the
    scalar-engine-bound pair-1 phase keeps the PE busy too.
  - DMA: criticality-ordered big pieces; issue split across the SP and
    Scalar sequencers (each dma_start costs ~0.6us of sequencer time).
  - memset-fed warmup matmuls bridge the DMA head so the PE never idles
    long enough for the HAM clock governor to re-throttle to 1.2GHz.
  - epilogue per head: copy denominator psum->sbuf, fast reciprocal,
    multiply straight out of psum (no [128,512] staging copy).
"""

import numpy as np
import ml_dtypes

import concourse.bass as bass
import concourse.tile as tile
from concourse import bacc, mybir
from concourse.bass_utils import run_bass_kernel_spmd

B, S, H, D = 2, 2048, 16, 64
HID = H * D
NCORES = 8
HPC = 4               # heads per core
COLS = HPC * D        # 256 projection columns per core
KC = HID // 128       # 8 contraction chunks for projections
QC = S // 512         # 4 query chunks of 512
MC = S // 128         # 16 key chunks of 128

BF16 = mybir.dt.bfloat16
F32 = mybir.dt.float32
np_bf16 = ml_dtypes.bfloat16

N_WARM = 24           # memset-fed filler matmuls bridging the DMA head

_CACHE = {}


def build(apply_mask: bool) -> bass.Bass:
    nc = bacc.Bacc(None, target_bir_lowering=False, debug=False)

    xT = nc.declare_dram_parameter("xT", [HID, S], BF16, isOutput=False)
    wq = nc.declare_dram_parameter("wq", [HID, COLS], BF16, isOutput=False)
    wk = nc.declare_dram_parameter("wk", [HID, COLS], BF16, isOutput=False)
    wv = nc.declare_dram_parameter("wv", [HID, COLS], BF16, isOutput=False)
    bq = nc.declare_dram_parameter("bq", [128, 2], F32, isOutput=False)
    bk = nc.declare_dram_parameter("bk", [128, 2], F32, isOutput=False)
    bv = nc.declare_dram_parameter("bv", [128, COLS], F32, isOutput=False)
    if apply_mask:
        mm_in = nc.declare_dram_parameter("maskm", [128, MC], F32, isOutput=False)
    out_ext = nc.declare_dram_parameter("out", [HPC, D, S], F32, isOutput=True)

    with tile.TileContext(nc) as tc:
        with (
            tc.tile_pool(name="singles", bufs=1) as singles,
            tc.tile_pool(name="work", bufs=4) as work,
            tc.tile_pool(name="psum", bufs=2, space="PSUM") as psum,
        ):
            wq_sb = singles.tile([128, KC, COLS], BF16)
            wk_sb = singles.tile([128, KC, COLS], BF16)
            wv_sb = singles.tile([128, KC, COLS], BF16)
            x_sb = singles.tile([128, KC, S], BF16)
            bq_sb = singles.tile([128, 2], F32)
            bk_sb = singles.tile([128, 2], F32)
            bv_sb = singles.tile([128, COLS], F32)
            if apply_mask:
                mm_sb = singles.tile([128, MC], F32)
            zt = singles.tile([128, 512], BF16)

            # ---- input DMA ----
            # SP sequencer: the critical chain (pair-0 weights, then x in
            # consumption order).  Each dma_start costs ~565ns of sequencer
            # time, so pieces are big; each lands on its own DMA queue.
            def w_piece(eng, dst, src, pcols, j, nk):
                # nk contraction chunks of a [HID, COLS] weight, col slice
                eng.dma_start(
                    out=dst[:, j * nk:(j + 1) * nk, pcols],
                    in_=src[j * nk * 128:(j + 1) * nk * 128, pcols]
                        .rearrange("(k p) c -> p k c", p=128))

            for j in range(2):
                w_piece(nc.sync, wk_sb, wk, slice(0, 128), j, 4)
            for j in range(2):
                w_piece(nc.sync, wq_sb, wq, slice(0, 128), j, 4)
            for qc in range(3):
                for kc in range(KC):
                    qsl = slice(qc * 512, (qc + 1) * 512)
                    nc.sync.dma_start(out=x_sb[:, kc, qsl],
                                      in_=xT[kc * 128:(kc + 1) * 128, qsl])
            for j in range(4):  # qc3 as 4 double-kc pieces
                nc.sync.dma_start(
                    out=x_sb[:, 2 * j:2 * j + 2, 1536:2048],
                    in_=xT[j * 256:(j + 1) * 256, 1536:2048]
                        .rearrange("(k p) s -> p k s", p=128))

            # Scalar sequencer: everything off the critical chain (biases,
            # wv, pair-1 weights).  Scalar is idle until the first exp at
            # ~20us, long after these issues drain.
            nc.scalar.dma_start(out=bk_sb, in_=bk[:, :])
            nc.scalar.dma_start(out=bq_sb, in_=bq[:, :])
            for j in range(4):
                w_piece(nc.scalar, wv_sb, wv, slice(0, COLS), j, 2)
            nc.scalar.dma_start(out=bv_sb, in_=bv[:, :])
            if apply_mask:
                nc.scalar.dma_start(out=mm_sb, in_=mm_in[:, :])
            for j in range(2):
                w_piece(nc.scalar, wk_sb, wk, slice(128, 256), j, 4)
            for j in range(2):
                w_piece(nc.scalar, wq_sb, wq, slice(128, 256), j, 4)

            # ---- static SBUF prep (DVE is idle during the DMA head) ----
            nc.vector.memset(zt, 0.0)
            # qTz zero halves are written once for the full S up front
            # (variant 0: head-b rows zero; variant 1: head-a rows zero)
            qTz = singles.tile([128, 2, 2, S], BF16)
            kT = singles.tile([128, 2, S], BF16)
            for p in range(2):
                nc.vector.memset(qTz[64:128, p, 0, :], 0.0)
                nc.vector.memset(qTz[0:64, p, 1, :], 0.0)
            # v_aug: [128, key_chunk, head, 128]; cols 64:128 are ones, so
            # the ctx matmul emits the softmax denominator into psum
            # partitions 64:128 at no extra cost (matmul cost is N-bound)
            v_aug = singles.tile([128, MC, HPC, 128], BF16)
            nc.vector.memset(v_aug[:, :, :, 64:128], 1.0)

            # ---- warmup ----
            # Filler matmuls on a zeroed tile: no DMA dependency, so the PE
            # starts at ~7us and stays busy until the first weight/x pieces
            # land (~15-18us).  Keeps the HAM activity window tripped so the
            # clock is at 2.4GHz when real work starts.
            warm_ps = psum.tile([128, 512], F32, tag="proj_ps", name="warm_ps")
            for i in range(N_WARM):
                nc.tensor.matmul(warm_ps, lhsT=zt[:, 0:128], rhs=zt,
                                 start=(i == 0), stop=(i == N_WARM - 1))

            # ---- projection helpers ----
            def project_T_qc(dst, w_sb, b_sb, p, qc, zpad=False):
                ps = psum.tile([128, 512], F32, tag="proj_ps", name=f"pt_{nc.next_id()}")
                for kc in range(KC):
                    nc.tensor.matmul(
                        ps,
                        lhsT=w_sb[:, kc, p * 128:(p + 1) * 128],
                        rhs=x_sb[:, kc, qc * 512:(qc + 1) * 512],
                        start=(kc == 0), stop=(kc == KC - 1),
                    )
                qsl = slice(qc * 512, (qc + 1) * 512)
                if zpad:
                    nc.vector.tensor_tensor(
                        out=dst[0:64, p, 0, qsl],
                        in0=ps[0:64, :],
                        in1=b_sb[0:64, p:p + 1].to_broadcast([64, 512]),
                        op=mybir.AluOpType.add,
                    )
                    nc.vector.tensor_tensor(
                        out=dst[64:128, p, 1, qsl],
                        in0=ps[64:128, :],
                        in1=b_sb[64:128, p:p + 1].to_broadcast([64, 512]),
                        op=mybir.AluOpType.add,
                    )
                else:
                    nc.vector.tensor_tensor(
                        out=dst[:, p, qsl],
                        in0=ps,
                        in1=b_sb[:, p:p + 1].to_broadcast([128, 512]),
                        op=mybir.AluOpType.add,
                    )

            def project_v_chunk(mc):
                ps = psum.tile([128, COLS], F32, tag="proj_ps", name=f"pv_{nc.next_id()}")
                for kc in range(KC):
                    nc.tensor.matmul(
                        ps,
                        lhsT=x_sb[:, kc, mc * 128:(mc + 1) * 128],
                        rhs=wv_sb[:, kc, :],
                        start=(kc == 0), stop=(kc == KC - 1),
                    )
                nc.vector.tensor_tensor(
                    out=v_aug[:, mc, :, 0:64],
                    in0=ps[:, :].rearrange("p (h d) -> p h d", h=HPC),
                    in1=bv_sb.rearrange("p (h d) -> p h d", h=HPC),
                    op=mybir.AluOpType.add,
                )
                if apply_mask:
                    nc.vector.tensor_tensor(
                        out=v_aug[:, mc, :, :],
                        in0=v_aug[:, mc, :, :],
                        in1=mm_sb[:, mc:mc + 1, None].to_broadcast([128, HPC, 128]),
                        op=mybir.AluOpType.mult,
                    )

            # Named projection groups: K{p}{j} = kT pair p keys j*512..,
            # Q{p}{j} = qTz pair p queries j*512.. (zero-padded variants).
            def K(p, j):
                return lambda: project_T_qc(kT, wk_sb, bk_sb, p, j)

            def Q(p, j):
                return lambda: project_T_qc(qTz, wq_sb, bq_sb, p, j, zpad=True)

            def V2(mc):
                return lambda: (project_v_chunk(mc), project_v_chunk(mc + 1))

            def attention(p, qc, hooks):
                """One (pair, query-chunk) block.  hooks: kc2 -> callable
                emitted between the score and ctx matmuls of that kc2."""
                ha, hb = 2 * p, 2 * p + 1
                qsl = slice(qc * 512, (qc + 1) * 512)
                ctx_a = psum.tile([128, 512], F32, tag="ctx", name=f"ca_{nc.next_id()}")
                ctx_b = psum.tile([128, 512], F32, tag="ctx", name=f"cb_{nc.next_id()}")
                for kc2 in range(MC // 2):
                    kc0, kc1 = 2 * kc2, 2 * kc2 + 1
                    s_a = psum.tile([128, 1024], F32, tag="sps", name=f"sa_{nc.next_id()}")
                    s_b = psum.tile([128, 1024], F32, tag="sps", name=f"sb_{nc.next_id()}")
                    # paired row-group matmuls: head a on array rows 0:63,
                    # head b on rows 64:127 run concurrently
                    for i, kc in enumerate((kc0, kc1)):
                        ksl = slice(kc * 128, (kc + 1) * 128)
                        csl = slice(i * 512, (i + 1) * 512)
                        nc.tensor.matmul(
                            s_a[:, csl], lhsT=kT[:, p, ksl], rhs=qTz[:, p, 0, qsl],
                            start=True, stop=True)
                        nc.tensor.matmul(
                            s_b[:, csl], lhsT=kT[:, p, ksl], rhs=qTz[:, p, 1, qsl],
                            start=True, stop=True)
                    hook = hooks.get(kc2)
                    if hook is not None:
                        hook()
                    e_a = work.tile([128, 1024], BF16, tag="expT", name=f"ea_{nc.next_id()}")
                    e_b = work.tile([128, 1024], BF16, tag="expT", name=f"eb_{nc.next_id()}")
                    nc.scalar.activation(e_a, s_a, mybir.ActivationFunctionType.Exp,
                                         scale=0.125)
                    nc.scalar.activation(e_b, s_b, mybir.ActivationFunctionType.Exp,
                                         scale=0.125)
                    for i, kc in enumerate((kc0, kc1)):
                        csl = slice(i * 512, (i + 1) * 512)
                        nc.tensor.matmul(
                            ctx_a, lhsT=v_aug[:, kc, ha, :], rhs=e_a[:, csl],
                            start=(kc == 0), stop=(kc == MC - 1))
                        nc.tensor.matmul(
                            ctx_b, lhsT=v_aug[:, kc, hb, :], rhs=e_b[:, csl],
                            start=(kc == 0), stop=(kc == MC - 1))
                for h, ctx in ((ha, ctx_a), (hb, ctx_b)):
                    # denominator (psum rows 64:128) -> sbuf base-0 tile for
                    # the custom-DVE reciprocal; multiply reads ctx psum
                    # directly so no staging copy of the whole tile
                    d0_sb = work.tile([64, 512], F32, tag="den0", name=f"d0_{nc.next_id()}")
                    nc.vector.tensor_copy(out=d0_sb, in_=ctx[64:128, :])
                    d_sb = work.tile([64, 512], F32, tag="den", name=f"d_{nc.next_id()}")
                    nc.vector.reciprocal_approx_fast(out=d_sb, in_=d0_sb)
                    o_sb = work.tile([64, 512], F32, tag="outt", name=f"o_{nc.next_id()}")
                    nc.vector.tensor_tensor(
                        out=o_sb, in0=ctx[0:64, :],
                        in1=d_sb,
                        op=mybir.AluOpType.mult)
                    for half in range(2):
                        osl = slice(half * 256, (half + 1) * 256)
                        nc.sync.dma_start(
                            out=out_ext[h][:, qc * 512 + half * 256:
                                           qc * 512 + (half + 1) * 256],
                            in_=o_sb[:, osl])

            # ---- block schedule ----
            # B1 must own kT0 (all keys), qT0 qc0, every v chunk, and the
            # Q01 prefetch; the other 10 groups go one or two per block so
            # blocks 6-8 (scalar-bound: 16 exps = 16.6us vs 13.7us of
            # attention matmuls) still keep the PE fed.
            project_T_qc(kT, wk_sb, bk_sb, 0, 0)          # K00
            project_T_qc(qTz, wq_sb, bq_sb, 0, 0, True)   # Q00
            project_v_chunk(0)
            project_v_chunk(1)

            b1_hooks = {0: V2(2), 1: lambda: (K(0, 1)(), V2(4)()),
                        2: V2(6), 3: lambda: (K(0, 2)(), V2(8)()),
                        4: V2(10), 5: lambda: (K(0, 3)(), V2(12)()),
                        6: V2(14), 7: Q(0, 1)}
            attention(0, 0, b1_hooks)
            attention(0, 1, {1: Q(0, 2), 5: K(1, 0)})
            attention(0, 2, {1: Q(0, 3), 5: K(1, 1)})
            attention(0, 3, {1: Q(1, 0), 5: K(1, 2)})
            attention(1, 0, {1: Q(1, 1), 5: K(1, 3)})
            attention(1, 1, {1: Q(1, 2)})
            attention(1, 2, {1: Q(1, 3)})
            attention(1, 3, {})

    nc.compile()
    return nc


def _get_nc(apply_mask: bool) -> bass.Bass:
    if apply_mask not in _CACHE:
        _CACHE[apply_mask] = build(apply_mask)
    return _CACHE[apply_mask]


def _in_maps(x, mask, Wq, bq, Wk, bk, Wv, bv, apply_mask):
    xT_b = [np.ascontiguousarray(x[b].T).astype(np_bf16) for b in range(B)]
    maps = []
    for c in range(NCORES):
        b, hg = c // 4, c % 4
        cs = slice(hg * COLS, (hg + 1) * COLS)
        m = {
            "xT": xT_b[b],
            "wq": np.ascontiguousarray(Wq[:, cs]).astype(np_bf16),
            "wk": np.ascontiguousarray(Wk[:, cs]).astype(np_bf16),
            "wv": np.ascontiguousarray(Wv[:, cs]).astype(np_bf16),
            "bq": np.ascontiguousarray(bq[cs].reshape(2, 128).T).astype(np.float32),
            "bk": np.ascontiguousarray(bk[cs].reshape(2, 128).T).astype(np.float32),
            "bv": np.ascontiguousarray(
                np.broadcast_to(bv[cs], (128, COLS))).astype(np.float32),
        }
        if apply_mask:
            m["maskm"] = np.ascontiguousarray(
                mask[b].astype(np.float32).reshape(MC, 128).T)
        maps.append(m)
    return maps


def _ensure_ntff_hook():
    """The agent image's antenv lacks axon_hooks; synthesize it so
    run_bass_kernel_spmd(trace=True) can reach the axon NTFF profiler."""
    import sys as _sys
    import types as _types
    try:
        from antenv import axon_hooks  # noqa: F401
        return
    except ImportError:
        pass
    import antenv
    mod = _types.ModuleType("antenv.axon_hooks")
    _hook = [None]
    mod.set_axon_ntff_profile_hook = lambda h: _hook.__setitem__(0, h)
    mod.get_axon_ntff_profile_hook = lambda: _hook[0]
    _sys.modules["antenv.axon_hooks"] = mod
    antenv.axon_hooks = mod
    from trn_agent_boot.trn_boot import _ntff_profile_via_ctypes
    mod.set_axon_ntff_profile_hook(
        _ntff_profile_via_ctypes("/opt/axon/libaxon_pjrt.so"))


def run(inputs: dict, trace: bool = False):
    if trace:
        _ensure_ntff_hook()
    x = np.asarray(inputs["x"], dtype=np.float32)
    mask = np.asarray(inputs["mask"])
    apply_mask = not bool((mask == 1).all())
    nc = _get_nc(apply_mask)
    maps = _in_maps(x, mask, np.asarray(inputs["Wq"], np.float32),
                    np.asarray(inputs["bq"], np.float32),
                    np.asarray(inputs["Wk"], np.float32),
                    np.asarray(inputs["bk"], np.float32),
                    np.asarray(inputs["Wv"], np.float32),
                    np.asarray(inputs["bv"], np.float32), apply_mask)
    res = run_bass_kernel_spmd(nc, maps, core_ids=list(range(NCORES)), trace=trace)
    out = np.empty((B, S, HID), dtype=np.float32)
    for c in range(NCORES):
        b, hg = c // 4, c % 4
        cs = slice(hg * COLS, (hg + 1) * COLS)
        ctxT = res.results[c]["out"]          # [HPC, D, S]
        out[b, :, cs] = ctxT.transpose(2, 0, 1).reshape(S, COLS)
    return out, res


def kernel(**inputs) -> np.ndarray:
    out, _ = run(inputs)
    return out


# revision 6
# speedup vs baseline: 1.1205x; 1.0484x over previous
"""Multi-head attention (B=2, S=2048, H=16, D=64) on 8 TRN2 NeuronCores.

Sharding: data parallel on batch (2) x tensor parallel on heads (16 -> 4 per
core).  Core c handles batch c//4 and heads [4*(c%4), 4*(c%4)+4).  Each core
projects q/k/v for its head group from its batch's activations, runs the
full S x S attention for its 4 heads, and writes ctx in [head, D, S] layout.
The host transposes/concatenates shards (not part of HW exec time).

Device kernel (per core, identical SPMD program, no collectives):
  - qT/kT computed directly in [D, S] layout (head pairs packed into 128
    partitions) so the scores matmul needs no transposes.
  - scoresT tiles [S_k=128, S_q] = kT_chunk.T @ qTz; softmax denominator via
    ones columns appended to v (one matmul stream produces ctx and denom).
  - exp on the scalar engine with the 1/sqrt(D) scale folded in; bf16
    matmul operands, f32 PSUM accumulation; final normalize = DVE divide.
  - padding mask folded into v_aug row zeroing (exp(x-1e4) underflows to 0
    in f32, so zeroing masked key rows is exactly equivalent).

v2 scheduling (vs the first working version):
  - 8 blocks B(p,qc); block 1 carries only its mandatory projections
    (kT0 all keys, qT0 qc0, all v chunks, plus Q01 prefetch); the
    remaining 10 projection groups are spread one or two per block so the
    scalar-engine-bound pair-1 phase keeps the PE busy too.
  - DMA: criticality-ordered big pieces; issue split across the SP and
    Scalar sequencers (each dma_start costs ~0.6us of sequencer time).
  - memset-fed warmup matmuls bridge the DMA head so the PE never idles
    long enough for the HAM clock governor to re-throttle to 1.2GHz.
  - epilogue per head: copy denominator psum->sbuf, fast reciprocal,
    multiply straight out of psum (no [128,512] staging copy).
"""

import numpy as np
import ml_dtypes

import concourse.bass as bass
import concourse.tile as tile
from concourse import bacc, mybir
from concourse.bass_utils import run_bass_kernel_spmd

B, S, H, D = 2, 2048, 16, 64
HID = H * D
NCORES = 8
HPC = 4               # heads per core
COLS = HPC * D        # 256 projection columns per core
KC = HID // 128       # 8 contraction chunks for projections
QC = S // 512         # 4 query chunks of 512
MC = S // 128         # 16 key chunks of 128

BF16 = mybir.dt.bfloat16
F32 = mybir.dt.float32
np_bf16 = ml_dtypes.bfloat16

N_WARM = 24           # memset-fed filler matmuls bridging the DMA head

_CACHE = {}


def build(apply_mask: bool) -> bass.Bass:
    nc = bacc.Bacc(None, target_bir_lowering=False, debug=False)

    xT = nc.declare_dram_parameter("xT", [HID, S], BF16, isOutput=False)
    wq = nc.declare_dram_parameter("wq", [HID, COLS], BF16, isOutput=False)
    wk = nc.declare_dram_parameter("wk", [HID, COLS], BF16, isOutput=False)
    wv = nc.declare_dram_parameter("wv", [HID, COLS], BF16, isOutput=False)
    bq = nc.declare_dram_parameter("bq", [128, 2], F32, isOutput=False)
    bk = nc.declare_dram_parameter("bk", [128, 2], F32, isOutput=False)
    bv = nc.declare_dram_parameter("bv", [128, COLS], F32, isOutput=False)
    if apply_mask:
        mm_in = nc.declare_dram_parameter("maskm", [128, MC], F32, isOutput=False)
    out_ext = nc.declare_dram_parameter("out", [HPC, D, S], F32, isOutput=True)

    with tile.TileContext(nc) as tc:
        with (
            tc.tile_pool(name="singles", bufs=1) as singles,
            tc.tile_pool(name="work", bufs=4) as work,
            tc.tile_pool(name="psum", bufs=2, space="PSUM") as psum,
        ):
            wq_sb = singles.tile([128, KC, COLS], BF16)
            wk_sb = singles.tile([128, KC, COLS], BF16)
            wv_sb = singles.tile([128, KC, COLS], BF16)
            x_sb = singles.tile([128, KC, S], BF16)
            bq_sb = singles.tile([128, 2], F32)
            bk_sb = singles.tile([128, 2], F32)
            bv_sb = singles.tile([128, COLS], F32)
            if apply_mask:
                mm_sb = singles.tile([128, MC], F32)
            zt = singles.tile([128, 512], BF16)

            # ---- input DMA ----
            # SP sequencer: the critical chain (pair-0 weights, then x in
            # consumption order).  Each dma_start costs ~565ns of sequencer
            # time, so pieces are big; each lands on its own DMA queue.
            def w_piece(eng, dst, src, pcols, j, nk):
                # nk contraction chunks of a [HID, COLS] weight, col slice
                eng.dma_start(
                    out=dst[:, j * nk:(j + 1) * nk, pcols],
                    in_=src[j * nk * 128:(j + 1) * nk * 128, pcols]
                        .rearrange("(k p) c -> p k c", p=128))

            for j in range(2):
                w_piece(nc.sync, wk_sb, wk, slice(0, 128), j, 4)
            for j in range(2):
                w_piece(nc.sync, wq_sb, wq, slice(0, 128), j, 4)
            for qc in range(3):
                for kc in range(KC):
                    qsl = slice(qc * 512, (qc + 1) * 512)
                    nc.sync.dma_start(out=x_sb[:, kc, qsl],
                                      in_=xT[kc * 128:(kc + 1) * 128, qsl])
            for j in range(4):  # qc3 as 4 double-kc pieces
                nc.sync.dma_start(
                    out=x_sb[:, 2 * j:2 * j + 2, 1536:2048],
                    in_=xT[j * 256:(j + 1) * 256, 1536:2048]
                        .rearrange("(k p) s -> p k s", p=128))

            # Scalar sequencer: everything off the critical chain (biases,
            # wv, pair-1 weights).  Scalar is idle until the first exp at
            # ~20us, long after these issues drain.
            nc.scalar.dma_start(out=bk_sb, in_=bk[:, :])
            nc.scalar.dma_start(out=bq_sb, in_=bq[:, :])
            for j in range(4):
                w_piece(nc.scalar, wv_sb, wv, slice(0, COLS), j, 2)
            nc.scalar.dma_start(out=bv_sb, in_=bv[:, :])
            if apply_mask:
                nc.scalar.dma_start(out=mm_sb, in_=mm_in[:, :])
            for j in range(2):
                w_piece(nc.scalar, wk_sb, wk, slice(128, 256), j, 4)
            for j in range(2):
                w_piece(nc.scalar, wq_sb, wq, slice(128, 256), j, 4)

            # ---- static SBUF prep (DVE is idle during the DMA head) ----
            nc.vector.memset(zt, 0.0)
            # qTz zero halves are written once for the full S up front
            # (variant 0: head-b rows zero; variant 1: head-a rows zero)
            qTz = singles.tile([128, 2, 2, S], BF16)
            kT = singles.tile([128, 2, S], BF16)
            for p in range(2):
                nc.vector.memset(qTz[64:128, p, 0, :], 0.0)
                nc.vector.memset(qTz[0:64, p, 1, :], 0.0)
            # v_aug: [128, key_chunk, head, 128]; cols 64:128 are ones, so
            # the ctx matmul emits the softmax denominator into psum
            # partitions 64:128 at no extra cost (matmul cost is N-bound)
            v_aug = singles.tile([128, MC, HPC, 128], BF16)
            nc.vector.memset(v_aug[:, :, :, 64:128], 1.0)

            # ---- projection helpers ----
            def project_T_qc(dst, w_sb, b_sb, p, qc, zpad=False):
                ps = psum.tile([128, 512], F32, tag="proj_ps", name=f"pt_{nc.next_id()}")
                for kc in range(KC):
                    nc.tensor.matmul(
                        ps,
                        lhsT=w_sb[:, kc, p * 128:(p + 1) * 128],
                        rhs=x_sb[:, kc, qc * 512:(qc + 1) * 512],
                        start=(kc == 0), stop=(kc == KC - 1),
                    )
                qsl = slice(qc * 512, (qc + 1) * 512)
                if zpad:
                    nc.vector.tensor_tensor(
                        out=dst[0:64, p, 0, qsl],
                        in0=ps[0:64, :],
                        in1=b_sb[0:64, p:p + 1].to_broadcast([64, 512]),
                        op=mybir.AluOpType.add,
                    )
                    nc.vector.tensor_tensor(
                        out=dst[64:128, p, 1, qsl],
                        in0=ps[64:128, :],
                        in1=b_sb[64:128, p:p + 1].to_broadcast([64, 512]),
                        op=mybir.AluOpType.add,
                    )
                else:
                    nc.vector.tensor_tensor(
                        out=dst[:, p, qsl],
                        in0=ps,
                        in1=b_sb[:, p:p + 1].to_broadcast([128, 512]),
                        op=mybir.AluOpType.add,
                    )

            def project_v_chunk(mc):
                ps = psum.tile([128, COLS], F32, tag="proj_ps", name=f"pv_{nc.next_id()}")
                for kc in range(KC):
                    nc.tensor.matmul(
                        ps,
                        lhsT=x_sb[:, kc, mc * 128:(mc + 1) * 128],
                        rhs=wv_sb[:, kc, :],
                        start=(kc == 0), stop=(kc == KC - 1),
                    )
                nc.vector.tensor_tensor(
                    out=v_aug[:, mc, :, 0:64],
                    in0=ps[:, :].rearrange("p (h d) -> p h d", h=HPC),
                    in1=bv_sb.rearrange("p (h d) -> p h d", h=HPC),
                    op=mybir.AluOpType.add,
                )
                if apply_mask:
                    nc.vector.tensor_tensor(
                        out=v_aug[:, mc, :, :],
                        in0=v_aug[:, mc, :, :],
                        in1=mm_sb[:, mc:mc + 1, None].to_broadcast([128, HPC, 128]),
                        op=mybir.AluOpType.mult,
                    )

            # Named projection groups: K{p}{j} = kT pair p keys j*512..,
            # Q{p}{j} = qTz pair p queries j*512.. (zero-padded variants).
            def K(p, j):
                return lambda: project_T_qc(kT, wk_sb, bk_sb, p, j)

            def Q(p, j):
                return lambda: project_T_qc(qTz, wq_sb, bq_sb, p, j, zpad=True)

            def V2(mc):
                return lambda: (project_v_chunk(mc), project_v_chunk(mc + 1))

            def emit_ctx(pend):
                p, qc, kc2, e_a, e_b, ctx_a, ctx_b = pend
                ha, hb = 2 * p, 2 * p + 1
                for i, kc in enumerate((2 * kc2, 2 * kc2 + 1)):
                    csl = slice(i * 512, (i + 1) * 512)
                    nc.tensor.matmul(
                        ctx_a, lhsT=v_aug[:, kc, ha, :], rhs=e_a[:, csl],
                        start=(kc == 0), stop=(kc == MC - 1))
                    nc.tensor.matmul(
                        ctx_b, lhsT=v_aug[:, kc, hb, :], rhs=e_b[:, csl],
                        start=(kc == 0), stop=(kc == MC - 1))

            def emit_epilogue(p, qc, ctx_a, ctx_b, last=False):
                # denominator (psum rows 64:128) -> sbuf base-0 tile for the
                # custom-DVE reciprocal; multiply reads ctx psum directly so
                # there is no staging copy of the whole tile
                for h, ctx in ((2 * p, ctx_a), (2 * p + 1, ctx_b)):
                    d0_sb = work.tile([64, 512], F32, tag="den0", name=f"d0_{nc.next_id()}")
                    nc.vector.tensor_copy(out=d0_sb, in_=ctx[64:128, :])
                    d_sb = work.tile([64, 512], F32, tag="den", name=f"d_{nc.next_id()}")
                    nc.vector.reciprocal_approx_fast(out=d_sb, in_=d0_sb)
                    o_sb = work.tile([64, 512], F32, tag="outt", name=f"o_{nc.next_id()}")
                    nc.vector.tensor_tensor(
                        out=o_sb, in0=ctx[0:64, :],
                        in1=d_sb,
                        op=mybir.AluOpType.mult)
                    # last block: 4-way DMA split so the final transfer
                    # isn't a single-queue 64KB straggler in the tail
                    nsplit = 4 if last else 2
                    for piece in range(nsplit):
                        w = 512 // nsplit
                        nc.sync.dma_start(
                            out=out_ext[h][:, qc * 512 + piece * w:
                                           qc * 512 + (piece + 1) * w],
                            in_=o_sb[:, piece * w:(piece + 1) * w])

            # ---- block schedule ----
            # B1 must own kT0 (all keys), qT0 qc0, every v chunk, and the
            # Q01 prefetch; the other 10 groups go one or two per block so
            # blocks 6-8 (scalar-bound: 16 exps = 16.6us vs 13.7us of
            # attention matmuls) still keep the PE fed.
            blocks = [(0, 0), (0, 1), (0, 2), (0, 3),
                      (1, 0), (1, 1), (1, 2), (1, 3)]
            hooks = [
                {0: V2(2), 1: lambda: (K(0, 1)(), V2(4)()),
                 2: V2(6), 3: lambda: (K(0, 2)(), V2(8)()),
                 4: V2(10), 5: lambda: (K(0, 3)(), V2(12)()),
                 6: V2(14), 7: Q(0, 1)},
                {1: Q(0, 2), 5: K(1, 0)},
                {1: Q(0, 3), 5: K(1, 1)},
                {1: Q(1, 0), 5: K(1, 2)},
                {1: Q(1, 1), 5: K(1, 3)},
                {1: Q(1, 2)},
                {1: Q(1, 3)},
                {},
            ]

            # ---- warmup + first projections ----
            # Filler matmuls on a zeroed tile bridge the DMA head (PE can
            # start at ~7us; first weight/x pieces land at ~13-15us), and
            # more fillers are interleaved into the DMA-arrival-paced K00 /
            # Q00 chains so the PE never idles long enough for the HAM
            # clock governor to drop back to 1.2GHz.
            warm_ps = psum.tile([128, 512], F32, tag="proj_ps", name="warm_ps")
            k00_ps = psum.tile([128, 512], F32, tag="proj_ps", name="k00_ps")
            nwarm = [0]

            def warm(n):
                for _ in range(n):
                    nc.tensor.matmul(warm_ps, lhsT=zt[:, 0:128], rhs=zt,
                                     start=(nwarm[0] == 0), stop=False)
                    nwarm[0] += 1

            warm(N_WARM)
            for kc in range(KC):
                nc.tensor.matmul(
                    k00_ps, lhsT=wk_sb[:, kc, 0:128],
                    rhs=x_sb[:, kc, 0:512],
                    start=(kc == 0), stop=(kc == KC - 1))
                warm(2)
            nc.vector.tensor_tensor(
                out=kT[:, 0, 0:512], in0=k00_ps,
                in1=bk_sb[:, 0:1].to_broadcast([128, 512]),
                op=mybir.AluOpType.add)
            # close the warm accumulation group (releases its psum slot)
            nc.tensor.matmul(warm_ps, lhsT=zt[:, 0:128], rhs=zt,
                             start=False, stop=True)
            project_T_qc(qTz, wq_sb, bq_sb, 0, 0, True)   # Q00
            project_v_chunk(0)
            project_v_chunk(1)

            # ---- software-pipelined attention ----
            # ctx matmuls lag one (block, kc2) step behind the score
            # matmuls, across block boundaries, so the PE computes the next
            # scores while the scalar engine is still on the previous exps
            # (the scalar engine is the per-block bottleneck for pair 1).
            pend = None
            for bi, (p, qc) in enumerate(blocks):
                qsl = slice(qc * 512, (qc + 1) * 512)
                ctx_a = psum.tile([128, 512], F32, tag="ctx", name=f"ca_{nc.next_id()}")
                ctx_b = psum.tile([128, 512], F32, tag="ctx", name=f"cb_{nc.next_id()}")
                for kc2 in range(MC // 2):
                    kc0, kc1 = 2 * kc2, 2 * kc2 + 1
                    s_a = psum.tile([128, 1024], F32, tag="sps", name=f"sa_{nc.next_id()}")
                    s_b = psum.tile([128, 1024], F32, tag="sps", name=f"sb_{nc.next_id()}")
                    # paired row-group matmuls: head a on array rows 0:63,
                    # head b on rows 64:127 run concurrently
                    for i, kc in enumerate((kc0, kc1)):
                        ksl = slice(kc * 128, (kc + 1) * 128)
                        csl = slice(i * 512, (i + 1) * 512)
                        nc.tensor.matmul(
                            s_a[:, csl], lhsT=kT[:, p, ksl], rhs=qTz[:, p, 0, qsl],
                            start=True, stop=True)
                        nc.tensor.matmul(
                            s_b[:, csl], lhsT=kT[:, p, ksl], rhs=qTz[:, p, 1, qsl],
                            start=True, stop=True)
                    hook = hooks[bi].get(kc2)
                    if hook is not None:
                        hook()
                    e_a = work.tile([128, 1024], BF16, tag="expT", name=f"ea_{nc.next_id()}")
                    e_b = work.tile([128, 1024], BF16, tag="expT", name=f"eb_{nc.next_id()}")
                    nc.scalar.activation(e_a, s_a, mybir.ActivationFunctionType.Exp,
                                         scale=0.125)
                    nc.scalar.activation(e_b, s_b, mybir.ActivationFunctionType.Exp,
                                         scale=0.125)
                    if pend is not None:
                        emit_ctx(pend)
                        if pend[2] == MC // 2 - 1:
                            emit_epilogue(pend[0], pend[1], pend[5], pend[6])
                    pend = (p, qc, kc2, e_a, e_b, ctx_a, ctx_b)
            emit_ctx(pend)
            emit_epilogue(pend[0], pend[1], pend[5], pend[6], last=True)

    nc.compile()
    return nc


def _get_nc(apply_mask: bool) -> bass.Bass:
    if apply_mask not in _CACHE:
        _CACHE[apply_mask] = build(apply_mask)
    return _CACHE[apply_mask]


def _in_maps(x, mask, Wq, bq, Wk, bk, Wv, bv, apply_mask):
    xT_b = [np.ascontiguousarray(x[b].T).astype(np_bf16) for b in range(B)]
    maps = []
    for c in range(NCORES):
        b, hg = c // 4, c % 4
        cs = slice(hg * COLS, (hg + 1) * COLS)
        m = {
            "xT": xT_b[b],
            "wq": np.ascontiguousarray(Wq[:, cs]).astype(np_bf16),
            "wk": np.ascontiguousarray(Wk[:, cs]).astype(np_bf16),
            "wv": np.ascontiguousarray(Wv[:, cs]).astype(np_bf16),
            "bq": np.ascontiguousarray(bq[cs].reshape(2, 128).T).astype(np.float32),
            "bk": np.ascontiguousarray(bk[cs].reshape(2, 128).T).astype(np.float32),
            "bv": np.ascontiguousarray(
                np.broadcast_to(bv[cs], (128, COLS))).astype(np.float32),
        }
        if apply_mask:
            m["maskm"] = np.ascontiguousarray(
                mask[b].astype(np.float32).reshape(MC, 128).T)
        maps.append(m)
    return maps


def _ensure_ntff_hook():
    """The agent image's antenv lacks axon_hooks; synthesize it so
    run_bass_kernel_spmd(trace=True) can reach the axon NTFF profiler."""
    import sys as _sys
    import types as _types
    try:
        from antenv import axon_hooks  # noqa: F401
        return
    except ImportError:
        pass
    import antenv
    mod = _types.ModuleType("antenv.axon_hooks")
    _hook = [None]
    mod.set_axon_ntff_profile_hook = lambda h: _hook.__setitem__(0, h)
    mod.get_axon_ntff_profile_hook = lambda: _hook[0]
    _sys.modules["antenv.axon_hooks"] = mod
    antenv.axon_hooks = mod
    from trn_agent_boot.trn_boot import _ntff_profile_via_ctypes
    mod.set_axon_ntff_profile_hook(
        _ntff_profile_via_ctypes("/opt/axon/libaxon_pjrt.so"))


def run(inputs: dict, trace: bool = False):
    if trace:
        _ensure_ntff_hook()
    x = np.asarray(inputs["x"], dtype=np.float32)
    mask = np.asarray(inputs["mask"])
    apply_mask = not bool((mask == 1).all())
    nc = _get_nc(apply_mask)
    maps = _in_maps(x, mask, np.asarray(inputs["Wq"], np.float32),
                    np.asarray(inputs["bq"], np.float32),
                    np.asarray(inputs["Wk"], np.float32),
                    np.asarray(inputs["bk"], np.float32),
                    np.asarray(inputs["Wv"], np.float32),
                    np.asarray(inputs["bv"], np.float32), apply_mask)
    res = run_bass_kernel_spmd(nc, maps, core_ids=list(range(NCORES)), trace=trace)
    out = np.empty((B, S, HID), dtype=np.float32)
    for c in range(NCORES):
        b, hg = c // 4, c % 4
        cs = slice(hg * COLS, (hg + 1) * COLS)
        ctxT = res.results[c]["out"]          # [HPC, D, S]
        out[b, :, cs] = ctxT.transpose(2, 0, 1).reshape(S, COLS)
    return out, res


def kernel(**inputs) -> np.ndarray:
    out, _ = run(inputs)
    return out


# revision 9
# speedup vs baseline: 1.1276x; 1.0063x over previous
"""Multi-head attention (B=2, S=2048, H=16, D=64) on 8 TRN2 NeuronCores.

Sharding: data parallel on batch (2) x tensor parallel on heads (16 -> 4 per
core).  Core c handles batch c//4 and heads [4*(c%4), 4*(c%4)+4).  Each core
projects q/k/v for its head group from its batch's activations, runs the
full S x S attention for its 4 heads, and writes ctx in [head, D, S] layout.
The host transposes/concatenates shards (not part of HW exec time).

Device kernel (per core, identical SPMD program, no collectives):
  - qT/kT computed directly in [D, S] layout (head pairs packed into 128
    partitions) so the scores matmul needs no transposes.
  - scoresT tiles [S_k=128, S_q] = kT_chunk.T @ qTz; softmax denominator via
    ones columns appended to v (one matmul stream produces ctx and denom).
  - exp on the scalar engine with the 1/sqrt(D) scale folded in; bf16
    matmul operands, f32 PSUM accumulation; final normalize = DVE divide.
  - padding mask folded into v_aug row zeroing (exp(x-1e4) underflows to 0
    in f32, so zeroing masked key rows is exactly equivalent).

v2 scheduling (vs the first working version):
  - 8 blocks B(p,qc); block 1 carries only its mandatory projections
    (kT0 all keys, qT0 qc0, all v chunks, plus Q01 prefetch); the
    remaining 10 projection groups are spread one or two per block so the
    scalar-engine-bound pair-1 phase keeps the PE busy too.
  - DMA: criticality-ordered big pieces; issue split across the SP and
    Scalar sequencers (each dma_start costs ~0.6us of sequencer time).
  - memset-fed warmup matmuls bridge the DMA head so the PE never idles
    long enough for the HAM clock governor to re-throttle to 1.2GHz.
  - epilogue per head: copy denominator psum->sbuf, fast reciprocal,
    multiply straight out of psum (no [128,512] staging copy).
"""

import numpy as np
import ml_dtypes

import concourse.bass as bass
import concourse.tile as tile
from concourse import bacc, mybir
from concourse.bass_utils import run_bass_kernel_spmd

B, S, H, D = 2, 2048, 16, 64
HID = H * D
NCORES = 8
HPC = 4               # heads per core
COLS = HPC * D        # 256 projection columns per core
KC = HID // 128       # 8 contraction chunks for projections
QC = S // 512         # 4 query chunks of 512
MC = S // 128         # 16 key chunks of 128

BF16 = mybir.dt.bfloat16
F32 = mybir.dt.float32
np_bf16 = ml_dtypes.bfloat16

N_WARM = 24           # memset-fed filler matmuls bridging the DMA head

_CACHE = {}


def build(apply_mask: bool) -> bass.Bass:
    nc = bacc.Bacc(None, target_bir_lowering=False, debug=False)

    xT = nc.declare_dram_parameter("xT", [HID, S], BF16, isOutput=False)
    wq = nc.declare_dram_parameter("wq", [HID, COLS], BF16, isOutput=False)
    wk = nc.declare_dram_parameter("wk", [HID, COLS], BF16, isOutput=False)
    wv = nc.declare_dram_parameter("wv", [HID, COLS], BF16, isOutput=False)
    bq = nc.declare_dram_parameter("bq", [128, 2], F32, isOutput=False)
    bk = nc.declare_dram_parameter("bk", [128, 2], F32, isOutput=False)
    bv = nc.declare_dram_parameter("bv", [128, COLS], F32, isOutput=False)
    if apply_mask:
        mm_in = nc.declare_dram_parameter("maskm", [128, MC], F32, isOutput=False)
    out_ext = nc.declare_dram_parameter("out", [HPC, D, S], F32, isOutput=True)

    with tile.TileContext(nc) as tc:
        with (
            tc.tile_pool(name="singles", bufs=1) as singles,
            tc.tile_pool(name="work", bufs=4) as work,
            tc.tile_pool(name="psum", bufs=2, space="PSUM") as psum,
        ):
            wq_sb = singles.tile([128, KC, COLS], BF16)
            wk_sb = singles.tile([128, KC, COLS], BF16)
            wv_sb = singles.tile([128, KC, COLS], BF16)
            x_sb = singles.tile([128, KC, S], BF16)
            bq_sb = singles.tile([128, 2], F32)
            bk_sb = singles.tile([128, 2], F32)
            bv_sb = singles.tile([128, COLS], F32)
            if apply_mask:
                mm_sb = singles.tile([128, MC], F32)
            zt = singles.tile([128, 512], BF16)

            # ---- input DMA ----
            # SP sequencer: the critical chain (pair-0 weights, then x in
            # consumption order).  Each dma_start costs ~565ns of sequencer
            # time, so pieces are big; each lands on its own DMA queue.
            def w_piece(eng, dst, src, pcols, j, nk):
                # nk contraction chunks of a [HID, COLS] weight, col slice
                eng.dma_start(
                    out=dst[:, j * nk:(j + 1) * nk, pcols],
                    in_=src[j * nk * 128:(j + 1) * nk * 128, pcols]
                        .rearrange("(k p) c -> p k c", p=128))

            def x_piece(eng, kc, qc):
                qsl = slice(qc * 512, (qc + 1) * 512)
                eng.dma_start(out=x_sb[:, kc, qsl],
                              in_=xT[kc * 128:(kc + 1) * 128, qsl])

            # x qc0 is the arrival-paced critical chain feeding K00/Q00:
            # alternate pieces between the SP and Scalar sequencers so the
            # ~0.6us per-issue cost doesn't serialize all eight.
            w_piece(nc.sync, wk_sb, wk, slice(0, 128), 0, 4)
            x_piece(nc.sync, 0, 0)
            x_piece(nc.sync, 2, 0)
            w_piece(nc.sync, wk_sb, wk, slice(0, 128), 1, 4)
            x_piece(nc.sync, 4, 0)
            x_piece(nc.sync, 6, 0)
            for j in range(2):
                w_piece(nc.sync, wq_sb, wq, slice(0, 128), j, 4)
            for qc in range(1, 3):
                for kc in range(KC):
                    x_piece(nc.sync, kc, qc)
            for j in range(4):  # qc3 as 4 double-kc pieces
                nc.sync.dma_start(
                    out=x_sb[:, 2 * j:2 * j + 2, 1536:2048],
                    in_=xT[j * 256:(j + 1) * 256, 1536:2048]
                        .rearrange("(k p) s -> p k s", p=128))

            # Scalar sequencer: the odd x qc0 pieces, then everything off
            # the critical chain (biases, wv, pair-1 weights).  Scalar is
            # idle until the first exp at ~19us, after these issues drain.
            for kc in (1, 3, 5, 7):
                x_piece(nc.scalar, kc, 0)
            nc.scalar.dma_start(out=bk_sb, in_=bk[:, :])
            nc.scalar.dma_start(out=bq_sb, in_=bq[:, :])
            for j in range(4):
                w_piece(nc.scalar, wv_sb, wv, slice(0, COLS), j, 2)
            nc.scalar.dma_start(out=bv_sb, in_=bv[:, :])
            if apply_mask:
                nc.scalar.dma_start(out=mm_sb, in_=mm_in[:, :])
            for j in range(2):
                w_piece(nc.scalar, wk_sb, wk, slice(128, 256), j, 4)
            for j in range(2):
                w_piece(nc.scalar, wq_sb, wq, slice(128, 256), j, 4)

            # ---- static SBUF prep (DVE is idle during the DMA head) ----
            nc.vector.memset(zt, 0.0)
            # qTz zero halves are written once for the full S up front
            # (variant 0: head-b rows zero; variant 1: head-a rows zero)
            qTz = singles.tile([128, 2, 2, S], BF16)
            kT = singles.tile([128, 2, S], BF16)
            for p in range(2):
                nc.vector.memset(qTz[64:128, p, 0, :], 0.0)
                nc.vector.memset(qTz[0:64, p, 1, :], 0.0)
            # v_aug: [128, key_chunk, head, 128]; cols 64:128 are ones, so
            # the ctx matmul emits the softmax denominator into psum
            # partitions 64:128 at no extra cost (matmul cost is N-bound)
            v_aug = singles.tile([128, MC, HPC, 128], BF16)
            nc.vector.memset(v_aug[:, :, :, 64:128], 1.0)

            # ---- projection helpers ----
            def project_T_qc(dst, w_sb, b_sb, p, qc, zpad=False):
                ps = psum.tile([128, 512], F32, tag="proj_ps", name=f"pt_{nc.next_id()}")
                for kc in range(KC):
                    nc.tensor.matmul(
                        ps,
                        lhsT=w_sb[:, kc, p * 128:(p + 1) * 128],
                        rhs=x_sb[:, kc, qc * 512:(qc + 1) * 512],
                        start=(kc == 0), stop=(kc == KC - 1),
                    )
                qsl = slice(qc * 512, (qc + 1) * 512)
                if zpad:
                    nc.vector.tensor_tensor(
                        out=dst[0:64, p, 0, qsl],
                        in0=ps[0:64, :],
                        in1=b_sb[0:64, p:p + 1].to_broadcast([64, 512]),
                        op=mybir.AluOpType.add,
                    )
                    nc.vector.tensor_tensor(
                        out=dst[64:128, p, 1, qsl],
                        in0=ps[64:128, :],
                        in1=b_sb[64:128, p:p + 1].to_broadcast([64, 512]),
                        op=mybir.AluOpType.add,
                    )
                else:
                    nc.vector.tensor_tensor(
                        out=dst[:, p, qsl],
                        in0=ps,
                        in1=b_sb[:, p:p + 1].to_broadcast([128, 512]),
                        op=mybir.AluOpType.add,
                    )

            def project_v_chunk(mc):
                ps = psum.tile([128, COLS], F32, tag="proj_ps", name=f"pv_{nc.next_id()}")
                for kc in range(KC):
                    nc.tensor.matmul(
                        ps,
                        lhsT=x_sb[:, kc, mc * 128:(mc + 1) * 128],
                        rhs=wv_sb[:, kc, :],
                        start=(kc == 0), stop=(kc == KC - 1),
                    )
                nc.vector.tensor_tensor(
                    out=v_aug[:, mc, :, 0:64],
                    in0=ps[:, :].rearrange("p (h d) -> p h d", h=HPC),
                    in1=bv_sb.rearrange("p (h d) -> p h d", h=HPC),
                    op=mybir.AluOpType.add,
                )
                if apply_mask:
                    nc.vector.tensor_tensor(
                        out=v_aug[:, mc, :, :],
                        in0=v_aug[:, mc, :, :],
                        in1=mm_sb[:, mc:mc + 1, None].to_broadcast([128, HPC, 128]),
                        op=mybir.AluOpType.mult,
                    )

            # Named projection groups: K{p}{j} = kT pair p keys j*512..,
            # Q{p}{j} = qTz pair p queries j*512.. (zero-padded variants).
            def K(p, j):
                return lambda: project_T_qc(kT, wk_sb, bk_sb, p, j)

            def Q(p, j):
                return lambda: project_T_qc(qTz, wq_sb, bq_sb, p, j, zpad=True)

            def V2(mc):
                return lambda: (project_v_chunk(mc), project_v_chunk(mc + 1))

            def emit_ctx(pend):
                p, qc, kc2, e_a, e_b, ctx_a, ctx_b = pend
                ha, hb = 2 * p, 2 * p + 1
                for i, kc in enumerate((2 * kc2, 2 * kc2 + 1)):
                    csl = slice(i * 512, (i + 1) * 512)
                    nc.tensor.matmul(
                        ctx_a, lhsT=v_aug[:, kc, ha, :], rhs=e_a[:, csl],
                        start=(kc == 0), stop=(kc == MC - 1))
                    nc.tensor.matmul(
                        ctx_b, lhsT=v_aug[:, kc, hb, :], rhs=e_b[:, csl],
                        start=(kc == 0), stop=(kc == MC - 1))

            def emit_epilogue(p, qc, ctx_a, ctx_b, last=False):
                # denominator (psum rows 64:128) -> sbuf base-0 tile for the
                # custom-DVE reciprocal; multiply reads ctx psum directly so
                # there is no staging copy of the whole tile
                for h, ctx in ((2 * p, ctx_a), (2 * p + 1, ctx_b)):
                    d0_sb = work.tile([64, 512], F32, tag="den0", name=f"d0_{nc.next_id()}")
                    nc.vector.tensor_copy(out=d0_sb, in_=ctx[64:128, :])
                    d_sb = work.tile([64, 512], F32, tag="den", name=f"d_{nc.next_id()}")
                    nc.vector.reciprocal_approx_fast(out=d_sb, in_=d0_sb)
                    o_sb = work.tile([64, 512], F32, tag="outt", name=f"o_{nc.next_id()}")
                    nc.vector.tensor_tensor(
                        out=o_sb, in0=ctx[0:64, :],
                        in1=d_sb,
                        op=mybir.AluOpType.mult)
                    # last block: 4-way DMA split issued from the Scalar
                    # sequencer, idle after the final exp (SP pays ~565ns
                    # per issue; a serial 64KB piece is a 2.8us straggler)
                    nsplit = 4 if last else 2
                    eng = nc.scalar if last else nc.sync
                    for piece in range(nsplit):
                        w = 512 // nsplit
                        eng.dma_start(
                            out=out_ext[h][:, qc * 512 + piece * w:
                                           qc * 512 + (piece + 1) * w],
                            in_=o_sb[:, piece * w:(piece + 1) * w])

            # ---- block schedule ----
            # B1 must own kT0 (all keys), qT0 qc0, every v chunk, and the
            # Q01 prefetch; the other 10 groups go one or two per block so
            # blocks 6-8 (scalar-bound: 16 exps = 16.6us vs 13.7us of
            # attention matmuls) still keep the PE fed.
            blocks = [(0, 0), (0, 1), (0, 2), (0, 3),
                      (1, 0), (1, 1), (1, 2), (1, 3)]
            hooks = [
                {0: V2(2), 1: lambda: (K(0, 1)(), V2(4)()),
                 2: V2(6), 3: lambda: (K(0, 2)(), V2(8)()),
                 4: V2(10), 5: lambda: (K(0, 3)(), V2(12)()),
                 6: V2(14), 7: Q(0, 1)},
                {1: Q(0, 2), 5: K(1, 0)},
                {1: Q(0, 3), 5: K(1, 1)},
                {1: Q(1, 0), 5: K(1, 2)},
                {1: Q(1, 1), 5: K(1, 3)},
                {1: Q(1, 2)},
                {1: Q(1, 3)},
                {},
            ]

            # ---- warmup + first projections ----
            # Filler matmuls on a zeroed tile bridge the DMA head (PE can
            # start at ~7us; first weight/x pieces land at ~13-15us), and
            # more fillers are interleaved into the DMA-arrival-paced K00 /
            # Q00 chains so the PE never idles long enough for the HAM
            # clock governor to drop back to 1.2GHz.
            warm_ps = psum.tile([128, 512], F32, tag="proj_ps", name="warm_ps")
            k00_ps = psum.tile([128, 512], F32, tag="proj_ps", name="k00_ps")
            nwarm = [0]

            def warm(n):
                for _ in range(n):
                    nc.tensor.matmul(warm_ps, lhsT=zt[:, 0:128], rhs=zt,
                                     start=(nwarm[0] == 0), stop=False)
                    nwarm[0] += 1

            warm(N_WARM)
            for kc in range(KC):
                nc.tensor.matmul(
                    k00_ps, lhsT=wk_sb[:, kc, 0:128],
                    rhs=x_sb[:, kc, 0:512],
                    start=(kc == 0), stop=(kc == KC - 1))
                warm(2)
            nc.vector.tensor_tensor(
                out=kT[:, 0, 0:512], in0=k00_ps,
                in1=bk_sb[:, 0:1].to_broadcast([128, 512]),
                op=mybir.AluOpType.add)
            # close the warm accumulation group (releases its psum slot)
            nc.tensor.matmul(warm_ps, lhsT=zt[:, 0:128], rhs=zt,
                             start=False, stop=True)
            project_T_qc(qTz, wq_sb, bq_sb, 0, 0, True)   # Q00
            project_v_chunk(0)
            project_v_chunk(1)

            # ---- software-pipelined attention ----
            # ctx matmuls lag one (block, kc2) step behind the score
            # matmuls, across block boundaries, so the PE computes the next
            # scores while the scalar engine is still on the previous exps
            # (the scalar engine is the per-block bottleneck for pair 1).
            pend = None
            for bi, (p, qc) in enumerate(blocks):
                qsl = slice(qc * 512, (qc + 1) * 512)
                ctx_a = psum.tile([128, 512], F32, tag="ctx", name=f"ca_{nc.next_id()}")
                ctx_b = psum.tile([128, 512], F32, tag="ctx", name=f"cb_{nc.next_id()}")
                for kc2 in range(MC // 2):
                    kc0, kc1 = 2 * kc2, 2 * kc2 + 1
                    s_a = psum.tile([128, 1024], F32, tag="sps", name=f"sa_{nc.next_id()}")
                    s_b = psum.tile([128, 1024], F32, tag="sps", name=f"sb_{nc.next_id()}")
                    # paired row-group matmuls: head a on array rows 0:63,
                    # head b on rows 64:127 run concurrently
                    for i, kc in enumerate((kc0, kc1)):
                        ksl = slice(kc * 128, (kc + 1) * 128)
                        csl = slice(i * 512, (i + 1) * 512)
                        nc.tensor.matmul(
                            s_a[:, csl], lhsT=kT[:, p, ksl], rhs=qTz[:, p, 0, qsl],
                            start=True, stop=True)
                        nc.tensor.matmul(
                            s_b[:, csl], lhsT=kT[:, p, ksl], rhs=qTz[:, p, 1, qsl],
                            start=True, stop=True)
                    hook = hooks[bi].get(kc2)
                    if hook is not None:
                        hook()
                    e_a = work.tile([128, 1024], BF16, tag="expT", name=f"ea_{nc.next_id()}")
                    e_b = work.tile([128, 1024], BF16, tag="expT", name=f"eb_{nc.next_id()}")
                    nc.scalar.activation(e_a, s_a, mybir.ActivationFunctionType.Exp,
                                         scale=0.125)
                    nc.scalar.activation(e_b, s_b, mybir.ActivationFunctionType.Exp,
                                         scale=0.125)
                    if pend is not None:
                        emit_ctx(pend)
                        if pend[2] == MC // 2 - 1:
                            emit_epilogue(pend[0], pend[1], pend[5], pend[6])
                    pend = (p, qc, kc2, e_a, e_b, ctx_a, ctx_b)
            emit_ctx(pend)
            emit_epilogue(pend[0], pend[1], pend[5], pend[6], last=True)

    nc.compile()
    return nc


def _get_nc(apply_mask: bool) -> bass.Bass:
    if apply_mask not in _CACHE:
        _CACHE[apply_mask] = build(apply_mask)
    return _CACHE[apply_mask]


def _in_maps(x, mask, Wq, bq, Wk, bk, Wv, bv, apply_mask):
    xT_b = [np.ascontiguousarray(x[b].T).astype(np_bf16) for b in range(B)]
    maps = []
    for c in range(NCORES):
        b, hg = c // 4, c % 4
        cs = slice(hg * COLS, (hg + 1) * COLS)
        m = {
            "xT": xT_b[b],
            "wq": np.ascontiguousarray(Wq[:, cs]).astype(np_bf16),
            "wk": np.ascontiguousarray(Wk[:, cs]).astype(np_bf16),
            "wv": np.ascontiguousarray(Wv[:, cs]).astype(np_bf16),
            "bq": np.ascontiguousarray(bq[cs].reshape(2, 128).T).astype(np.float32),
            "bk": np.ascontiguousarray(bk[cs].reshape(2, 128).T).astype(np.float32),
            "bv": np.ascontiguousarray(
                np.broadcast_to(bv[cs], (128, COLS))).astype(np.float32),
        }
        if apply_mask:
            m["maskm"] = np.ascontiguousarray(
                mask[b].astype(np.float32).reshape(MC, 128).T)
        maps.append(m)
    return maps


def _ensure_ntff_hook():
    """The agent image's antenv lacks axon_hooks; synthesize it so
    run_bass_kernel_spmd(trace=True) can reach the axon NTFF profiler."""
    import sys as _sys
    import types as _types
    try:
        from antenv import axon_hooks  # noqa: F401
        return
    except ImportError:
        pass
    import antenv
    mod = _types.ModuleType("antenv.axon_hooks")
    _hook = [None]
    mod.set_axon_ntff_profile_hook = lambda h: _hook.__setitem__(0, h)
    mod.get_axon_ntff_profile_hook = lambda: _hook[0]
    _sys.modules["antenv.axon_hooks"] = mod
    antenv.axon_hooks = mod
    from trn_agent_boot.trn_boot import _ntff_profile_via_ctypes
    mod.set_axon_ntff_profile_hook(
        _ntff_profile_via_ctypes("/opt/axon/libaxon_pjrt.so"))


def run(inputs: dict, trace: bool = False):
    if trace:
        _ensure_ntff_hook()
    x = np.asarray(inputs["x"], dtype=np.float32)
    mask = np.asarray(inputs["mask"])
    apply_mask = not bool((mask == 1).all())
    nc = _get_nc(apply_mask)
    maps = _in_maps(x, mask, np.asarray(inputs["Wq"], np.float32),
                    np.asarray(inputs["bq"], np.float32),
                    np.asarray(inputs["Wk"], np.float32),
                    np.asarray(inputs["bk"], np.float32),
                    np.asarray(inputs["Wv"], np.float32),
                    np.asarray(inputs["bv"], np.float32), apply_mask)
    res = run_bass_kernel_spmd(nc, maps, core_ids=list(range(NCORES)), trace=trace)
    out = np.empty((B, S, HID), dtype=np.float32)
    for c in range(NCORES):
        b, hg = c // 4, c % 4
        cs = slice(hg * COLS, (hg + 1) * COLS)
        ctxT = res.results[c]["out"]          # [HPC, D, S]
        out[b, :, cs] = ctxT.transpose(2, 0, 1).reshape(S, COLS)
    return out, res


def kernel(**inputs) -> np.ndarray:
    out, _ = run(inputs)
    return out


# revision 13
# speedup vs baseline: 1.1449x; 1.0153x over previous
"""Multi-head attention (B=2, S=2048, H=16, D=64) on 8 TRN2 NeuronCores.

Sharding: data parallel on batch (2) x tensor parallel on heads (16 -> 4 per
core).  Core c handles batch c//4 and heads [4*(c%4), 4*(c%4)+4).  Each core
projects q/k/v for its head group from its batch's activations, runs the
full S x S attention for its 4 heads, and writes ctx in [head, D, S] layout.
The host transposes/concatenates shards (not part of HW exec time).

Device kernel (per core, identical SPMD program, no collectives):
  - qT/kT computed directly in [D, S] layout (head pairs packed into 128
    partitions) so the scores matmul needs no transposes.
  - scoresT tiles [S_k=128, S_q] = kT_chunk.T @ qTz; softmax denominator via
    ones columns appended to v (one matmul stream produces ctx and denom).
  - exp on the scalar engine with the 1/sqrt(D) scale folded in; bf16
    matmul operands, f32 PSUM accumulation; final normalize = DVE divide.
  - padding mask folded into v_aug row zeroing (exp(x-1e4) underflows to 0
    in f32, so zeroing masked key rows is exactly equivalent).

v2 scheduling (vs the first working version):
  - 8 blocks B(p,qc); block 1 carries only its mandatory projections
    (kT0 all keys, qT0 qc0, all v chunks, plus Q01 prefetch); the
    remaining 10 projection groups are spread one or two per block so the
    scalar-engine-bound pair-1 phase keeps the PE busy too.
  - DMA: criticality-ordered big pieces; issue split across the SP and
    Scalar sequencers (each dma_start costs ~0.6us of sequencer time).
  - memset-fed warmup matmuls bridge the DMA head so the PE never idles
    long enough for the HAM clock governor to re-throttle to 1.2GHz.
  - epilogue per head: copy denominator psum->sbuf, fast reciprocal,
    multiply straight out of psum (no [128,512] staging copy).
"""

import numpy as np
import ml_dtypes

import concourse.bass as bass
import concourse.tile as tile
from concourse import bacc, mybir
from concourse.bass_utils import run_bass_kernel_spmd

B, S, H, D = 2, 2048, 16, 64
HID = H * D
NCORES = 8
HPC = 4               # heads per core
COLS = HPC * D        # 256 projection columns per core
KC = HID // 128       # 8 contraction chunks for projections
QC = S // 512         # 4 query chunks of 512
MC = S // 128         # 16 key chunks of 128

BF16 = mybir.dt.bfloat16
F32 = mybir.dt.float32
np_bf16 = ml_dtypes.bfloat16

N_WARM = 8            # memset-fed filler matmuls bridging the DMA head

_CACHE = {}


def build(apply_mask: bool) -> bass.Bass:
    nc = bacc.Bacc(None, target_bir_lowering=False, debug=False)

    xT = nc.declare_dram_parameter("xT", [HID, S], BF16, isOutput=False)
    wq = nc.declare_dram_parameter("wq", [HID, COLS], BF16, isOutput=False)
    wk = nc.declare_dram_parameter("wk", [HID, COLS], BF16, isOutput=False)
    wv = nc.declare_dram_parameter("wv", [HID, COLS], BF16, isOutput=False)
    bq = nc.declare_dram_parameter("bq", [128, 2], F32, isOutput=False)
    bk = nc.declare_dram_parameter("bk", [128, 2], F32, isOutput=False)
    bv = nc.declare_dram_parameter("bv", [128, COLS], F32, isOutput=False)
    if apply_mask:
        mm_in = nc.declare_dram_parameter("maskm", [128, MC], F32, isOutput=False)
    out_ext = nc.declare_dram_parameter("out", [HPC, D, S], F32, isOutput=True)

    with tile.TileContext(nc) as tc:
        with (
            tc.tile_pool(name="singles", bufs=1) as singles,
            tc.tile_pool(name="work", bufs=4) as work,
            tc.tile_pool(name="psum", bufs=2, space="PSUM") as psum,
        ):
            wq_sb = singles.tile([128, KC, COLS], BF16)
            wk_sb = singles.tile([128, KC, COLS], BF16)
            wv_sb = singles.tile([128, KC, COLS], BF16)
            x_sb = singles.tile([128, KC, S], BF16)
            bq_sb = singles.tile([128, 2], F32)
            bk_sb = singles.tile([128, 2], F32)
            bv_sb = singles.tile([128, COLS], F32)
            if apply_mask:
                mm_sb = singles.tile([128, MC], F32)
            zt = singles.tile([128, 512], BF16)

            # ---- input DMA ----
            # SP sequencer: the critical chain (pair-0 weights, then x in
            # consumption order).  Each dma_start costs ~565ns of sequencer
            # time, so pieces are big; each lands on its own DMA queue.
            def w_piece(eng, dst, src, pcols, j, nk):
                # nk contraction chunks of a [HID, COLS] weight, col slice
                eng.dma_start(
                    out=dst[:, j * nk:(j + 1) * nk, pcols],
                    in_=src[j * nk * 128:(j + 1) * nk * 128, pcols]
                        .rearrange("(k p) c -> p k c", p=128))

            def x_piece(eng, kc, qc):
                qsl = slice(qc * 512, (qc + 1) * 512)
                eng.dma_start(out=x_sb[:, kc, qsl],
                              in_=xT[kc * 128:(kc + 1) * 128, qsl])

            # x qc0 is the arrival-paced critical chain feeding K00/Q00:
            # alternate pieces between the SP and Scalar sequencers so the
            # ~0.6us per-issue cost doesn't serialize all eight.
            w_piece(nc.sync, wk_sb, wk, slice(0, 128), 0, 4)
            x_piece(nc.sync, 0, 0)
            x_piece(nc.sync, 2, 0)
            w_piece(nc.sync, wk_sb, wk, slice(0, 128), 1, 4)
            x_piece(nc.sync, 4, 0)
            x_piece(nc.sync, 6, 0)
            for j in range(2):
                w_piece(nc.sync, wq_sb, wq, slice(0, 128), j, 4)
            for qc in range(1, 3):
                for kc in range(KC):
                    x_piece(nc.sync, kc, qc)
            for j in range(4):  # qc3 as 4 double-kc pieces
                nc.sync.dma_start(
                    out=x_sb[:, 2 * j:2 * j + 2, 1536:2048],
                    in_=xT[j * 256:(j + 1) * 256, 1536:2048]
                        .rearrange("(k p) s -> p k s", p=128))

            # Scalar sequencer: the odd x qc0 pieces, then everything off
            # the critical chain (biases, wv, pair-1 weights).  Scalar is
            # idle until the first exp at ~19us, after these issues drain.
            for kc in (1, 3, 5, 7):
                x_piece(nc.scalar, kc, 0)
            nc.scalar.dma_start(out=bk_sb, in_=bk[:, :])
            nc.scalar.dma_start(out=bq_sb, in_=bq[:, :])
            for j in range(4):
                w_piece(nc.scalar, wv_sb, wv, slice(0, COLS), j, 2)
            nc.scalar.dma_start(out=bv_sb, in_=bv[:, :])
            if apply_mask:
                nc.scalar.dma_start(out=mm_sb, in_=mm_in[:, :])
            for j in range(2):
                w_piece(nc.scalar, wk_sb, wk, slice(128, 256), j, 4)
            for j in range(2):
                w_piece(nc.scalar, wq_sb, wq, slice(128, 256), j, 4)

            # ---- static SBUF prep (DVE is idle during the DMA head) ----
            nc.vector.memset(zt, 0.0)
            # qTz zero halves are written once for the full S up front
            # (variant 0: head-b rows zero; variant 1: head-a rows zero)
            qTz = singles.tile([128, 2, 2, S], BF16)
            kT = singles.tile([128, 2, S], BF16)
            for p in range(2):
                nc.vector.memset(qTz[64:128, p, 0, :], 0.0)
                nc.vector.memset(qTz[0:64, p, 1, :], 0.0)
            # v_aug: [128, key_chunk, head, 128]; cols 64:128 are ones, so
            # the ctx matmul emits the softmax denominator into psum
            # partitions 64:128 at no extra cost (matmul cost is N-bound)
            v_aug = singles.tile([128, MC, HPC, 128], BF16)
            nc.vector.memset(v_aug[:, :, :, 64:128], 1.0)

            # ---- projection helpers ----
            def project_T_qc(dst, w_sb, b_sb, p, qc, zpad=False):
                ps = psum.tile([128, 512], F32, tag="proj_ps", name=f"pt_{nc.next_id()}")
                for kc in range(KC):
                    nc.tensor.matmul(
                        ps,
                        lhsT=w_sb[:, kc, p * 128:(p + 1) * 128],
                        rhs=x_sb[:, kc, qc * 512:(qc + 1) * 512],
                        start=(kc == 0), stop=(kc == KC - 1),
                    )
                qsl = slice(qc * 512, (qc + 1) * 512)
                if zpad:
                    nc.vector.tensor_tensor(
                        out=dst[0:64, p, 0, qsl],
                        in0=ps[0:64, :],
                        in1=b_sb[0:64, p:p + 1].to_broadcast([64, 512]),
                        op=mybir.AluOpType.add,
                    )
                    nc.vector.tensor_tensor(
                        out=dst[64:128, p, 1, qsl],
                        in0=ps[64:128, :],
                        in1=b_sb[64:128, p:p + 1].to_broadcast([64, 512]),
                        op=mybir.AluOpType.add,
                    )
                else:
                    nc.vector.tensor_tensor(
                        out=dst[:, p, qsl],
                        in0=ps,
                        in1=b_sb[:, p:p + 1].to_broadcast([128, 512]),
                        op=mybir.AluOpType.add,
                    )

            def project_v_chunk(mc):
                ps = psum.tile([128, COLS], F32, tag="proj_ps", name=f"pv_{nc.next_id()}")
                for kc in range(KC):
                    nc.tensor.matmul(
                        ps,
                        lhsT=x_sb[:, kc, mc * 128:(mc + 1) * 128],
                        rhs=wv_sb[:, kc, :],
                        start=(kc == 0), stop=(kc == KC - 1),
                    )
                nc.vector.tensor_tensor(
                    out=v_aug[:, mc, :, 0:64],
                    in0=ps[:, :].rearrange("p (h d) -> p h d", h=HPC),
                    in1=bv_sb.rearrange("p (h d) -> p h d", h=HPC),
                    op=mybir.AluOpType.add,
                )
                if apply_mask:
                    nc.vector.tensor_tensor(
                        out=v_aug[:, mc, :, :],
                        in0=v_aug[:, mc, :, :],
                        in1=mm_sb[:, mc:mc + 1, None].to_broadcast([128, HPC, 128]),
                        op=mybir.AluOpType.mult,
                    )

            # Named projection groups: K{p}{j} = kT pair p keys j*512..,
            # Q{p}{j} = qTz pair p queries j*512.. (zero-padded variants).
            def K(p, j):
                return lambda: project_T_qc(kT, wk_sb, bk_sb, p, j)

            def Q(p, j):
                return lambda: project_T_qc(qTz, wq_sb, bq_sb, p, j, zpad=True)

            def V2(mc):
                return lambda: (project_v_chunk(mc), project_v_chunk(mc + 1))

            def emit_ctx(pend):
                p, qc, kc2, e_a, e_b, ctx_a, ctx_b = pend
                ha, hb = 2 * p, 2 * p + 1
                for i, kc in enumerate((2 * kc2, 2 * kc2 + 1)):
                    csl = slice(i * 512, (i + 1) * 512)
                    nc.tensor.matmul(
                        ctx_a, lhsT=v_aug[:, kc, ha, :], rhs=e_a[:, csl],
                        start=(kc == 0), stop=(kc == MC - 1))
                    nc.tensor.matmul(
                        ctx_b, lhsT=v_aug[:, kc, hb, :], rhs=e_b[:, csl],
                        start=(kc == 0), stop=(kc == MC - 1))

            def emit_epilogue(p, qc, ctx_a, ctx_b, last=False):
                # denominator (psum rows 64:128) -> sbuf base-0 tile for the
                # custom-DVE reciprocal; multiply reads ctx psum directly so
                # there is no staging copy of the whole tile.  In the last
                # block the copy runs on the Scalar engine (idle after the
                # final exp) and the out-DMA issues are split across the SP
                # and Scalar sequencers to shorten the tail.
                for h, ctx in ((2 * p, ctx_a), (2 * p + 1, ctx_b)):
                    d0_sb = work.tile([64, 512], F32, tag="den0", name=f"d0_{nc.next_id()}")
                    nc.vector.tensor_copy(out=d0_sb, in_=ctx[64:128, :])
                    d_sb = work.tile([64, 512], F32, tag="den", name=f"d_{nc.next_id()}")
                    nc.vector.reciprocal_approx_fast(out=d_sb, in_=d0_sb)
                    o_sb = work.tile([64, 512], F32, tag="outt", name=f"o_{nc.next_id()}")
                    nc.vector.tensor_tensor(
                        out=o_sb, in0=ctx[0:64, :],
                        in1=d_sb,
                        op=mybir.AluOpType.mult)
                    for piece in range(2):
                        eng = (nc.scalar if piece else nc.sync) if last else nc.sync
                        eng.dma_start(
                            out=out_ext[h][:, qc * 512 + piece * 256:
                                           qc * 512 + (piece + 1) * 256],
                            in_=o_sb[:, piece * 256:(piece + 1) * 256])

            # ---- block schedule ----
            # B1 must own kT0 (all keys), qT0 qc0, every v chunk, and the
            # Q01 prefetch; the other 10 groups go one or two per block so
            # blocks 6-8 (scalar-bound: 16 exps = 16.6us vs 13.7us of
            # attention matmuls) still keep the PE fed.
            blocks = [(0, 0), (0, 1), (0, 2), (0, 3),
                      (1, 0), (1, 1), (1, 2), (1, 3)]
            hooks = [
                {0: V2(2), 1: lambda: (K(0, 1)(), V2(4)()),
                 2: V2(6), 3: lambda: (K(0, 2)(), V2(8)()),
                 4: V2(10), 5: lambda: (K(0, 3)(), V2(12)()),
                 6: V2(14), 7: Q(0, 1)},
                {1: Q(0, 2), 5: K(1, 0)},
                {1: Q(0, 3), 5: K(1, 1)},
                {1: Q(1, 0), 5: K(1, 2)},
                {1: Q(1, 1), 5: K(1, 3)},
                {1: Q(1, 2)},
                {1: Q(1, 3)},
                {},
            ]

            # ---- warmup + first projections ----
            # Filler matmuls on a zeroed tile bridge the DMA head (PE can
            # start at ~7us; first weight/x pieces land at ~13-15us), and
            # more fillers are interleaved into the DMA-arrival-paced K00 /
            # Q00 chains so the PE never idles long enough for the HAM
            # clock governor to drop back to 1.2GHz.
            warm_ps = psum.tile([128, 512], F32, tag="proj_ps", name="warm_ps")
            k00_ps = psum.tile([128, 512], F32, tag="proj_ps", name="k00_ps")
            nwarm = [0]

            def warm(n):
                for _ in range(n):
                    nc.tensor.matmul(warm_ps, lhsT=zt[:, 0:128], rhs=zt,
                                     start=(nwarm[0] == 0), stop=False)
                    nwarm[0] += 1

            warm(N_WARM)
            for kc in range(KC):
                nc.tensor.matmul(
                    k00_ps, lhsT=wk_sb[:, kc, 0:128],
                    rhs=x_sb[:, kc, 0:512],
                    start=(kc == 0), stop=(kc == KC - 1))
            nc.vector.tensor_tensor(
                out=kT[:, 0, 0:512], in0=k00_ps,
                in1=bk_sb[:, 0:1].to_broadcast([128, 512]),
                op=mybir.AluOpType.add)
            # close the warm accumulation group (releases its psum slot)
            nc.tensor.matmul(warm_ps, lhsT=zt[:, 0:128], rhs=zt,
                             start=False, stop=True)
            project_T_qc(qTz, wq_sb, bq_sb, 0, 0, True)   # Q00
            project_v_chunk(0)
            project_v_chunk(1)

            # ---- software-pipelined attention ----
            # ctx matmuls lag one (block, kc2) step behind the score
            # matmuls, across block boundaries, so the PE computes the next
            # scores while the scalar engine is still on the previous exps
            # (the scalar engine is the per-block bottleneck for pair 1).
            pend = None
            for bi, (p, qc) in enumerate(blocks):
                qsl = slice(qc * 512, (qc + 1) * 512)
                ctx_a = psum.tile([128, 512], F32, tag="ctx", name=f"ca_{nc.next_id()}")
                ctx_b = psum.tile([128, 512], F32, tag="ctx", name=f"cb_{nc.next_id()}")
                for kc2 in range(MC // 2):
                    kc0, kc1 = 2 * kc2, 2 * kc2 + 1
                    s_a = psum.tile([128, 1024], F32, tag="sps", name=f"sa_{nc.next_id()}")
                    s_b = psum.tile([128, 1024], F32, tag="sps", name=f"sb_{nc.next_id()}")
                    # paired row-group matmuls: head a on array rows 0:63,
                    # head b on rows 64:127 run concurrently
                    for i, kc in enumerate((kc0, kc1)):
                        ksl = slice(kc * 128, (kc + 1) * 128)
                        csl = slice(i * 512, (i + 1) * 512)
                        nc.tensor.matmul(
                            s_a[:, csl], lhsT=kT[:, p, ksl], rhs=qTz[:, p, 0, qsl],
                            start=True, stop=True)
                        nc.tensor.matmul(
                            s_b[:, csl], lhsT=kT[:, p, ksl], rhs=qTz[:, p, 1, qsl],
                            start=True, stop=True)
                    hook = hooks[bi].get(kc2)
                    if hook is not None:
                        hook()
                    e_a = work.tile([128, 1024], BF16, tag="expT", name=f"ea_{nc.next_id()}")
                    e_b = work.tile([128, 1024], BF16, tag="expT", name=f"eb_{nc.next_id()}")
                    nc.scalar.activation(e_a, s_a, mybir.ActivationFunctionType.Exp,
                                         scale=0.125)
                    nc.scalar.activation(e_b, s_b, mybir.ActivationFunctionType.Exp,
                                         scale=0.125)
                    if pend is not None:
                        emit_ctx(pend)
                        if pend[2] == MC // 2 - 1:
                            emit_epilogue(pend[0], pend[1], pend[5], pend[6])
                    pend = (p, qc, kc2, e_a, e_b, ctx_a, ctx_b)
            emit_ctx(pend)
            emit_epilogue(pend[0], pend[1], pend[5], pend[6], last=True)

    nc.compile()
    return nc


def _get_nc(apply_mask: bool) -> bass.Bass:
    if apply_mask not in _CACHE:
        _CACHE[apply_mask] = build(apply_mask)
    return _CACHE[apply_mask]


def _in_maps(x, mask, Wq, bq, Wk, bk, Wv, bv, apply_mask):
    xT_b = [np.ascontiguousarray(x[b].T).astype(np_bf16) for b in range(B)]
    maps = []
    for c in range(NCORES):
        b, hg = c // 4, c % 4
        cs = slice(hg * COLS, (hg + 1) * COLS)
        m = {
            "xT": xT_b[b],
            "wq": np.ascontiguousarray(Wq[:, cs]).astype(np_bf16),
            "wk": np.ascontiguousarray(Wk[:, cs]).astype(np_bf16),
            "wv": np.ascontiguousarray(Wv[:, cs]).astype(np_bf16),
            "bq": np.ascontiguousarray(bq[cs].reshape(2, 128).T).astype(np.float32),
            "bk": np.ascontiguousarray(bk[cs].reshape(2, 128).T).astype(np.float32),
            "bv": np.ascontiguousarray(
                np.broadcast_to(bv[cs], (128, COLS))).astype(np.float32),
        }
        if apply_mask:
            m["maskm"] = np.ascontiguousarray(
                mask[b].astype(np.float32).reshape(MC, 128).T)
        maps.append(m)
    return maps


def _ensure_ntff_hook():
    """The agent image's antenv lacks axon_hooks; synthesize it so
    run_bass_kernel_spmd(trace=True) can reach the axon NTFF profiler."""
    import sys as _sys
    import types as _types
    try:
        from antenv import axon_hooks  # noqa: F401
        return
    except ImportError:
        pass
    import antenv
    mod = _types.ModuleType("antenv.axon_hooks")
    _hook = [None]
    mod.set_axon_ntff_profile_hook = lambda h: _hook.__setitem__(0, h)
    mod.get_axon_ntff_profile_hook = lambda: _hook[0]
    _sys.modules["antenv.axon_hooks"] = mod
    antenv.axon_hooks = mod
    from trn_agent_boot.trn_boot import _ntff_profile_via_ctypes
    mod.set_axon_ntff_profile_hook(
        _ntff_profile_via_ctypes("/opt/axon/libaxon_pjrt.so"))


def run(inputs: dict, trace: bool = False):
    if trace:
        _ensure_ntff_hook()
    x = np.asarray(inputs["x"], dtype=np.float32)
    mask = np.asarray(inputs["mask"])
    apply_mask = not bool((mask == 1).all())
    nc = _get_nc(apply_mask)
    maps = _in_maps(x, mask, np.asarray(inputs["Wq"], np.float32),
                    np.asarray(inputs["bq"], np.float32),
                    np.asarray(inputs["Wk"], np.float32),
                    np.asarray(inputs["bk"], np.float32),
                    np.asarray(inputs["Wv"], np.float32),
                    np.asarray(inputs["bv"], np.float32), apply_mask)
    res = run_bass_kernel_spmd(nc, maps, core_ids=list(range(NCORES)), trace=trace)
    out = np.empty((B, S, HID), dtype=np.float32)
    for c in range(NCORES):
        b, hg = c // 4, c % 4
        cs = slice(hg * COLS, (hg + 1) * COLS)
        ctxT = res.results[c]["out"]          # [HPC, D, S]
        out[b, :, cs] = ctxT.transpose(2, 0, 1).reshape(S, COLS)
    return out, res


def kernel(**inputs) -> np.ndarray:
    out, _ = run(inputs)
    return out


# revision 17
# speedup vs baseline: 1.1499x; 1.0044x over previous
"""Multi-head attention (B=2, S=2048, H=16, D=64) on 8 TRN2 NeuronCores.

Sharding: data parallel on batch (2) x tensor parallel on heads (16 -> 4 per
core).  Core c handles batch c//4 and heads [4*(c%4), 4*(c%4)+4).  Each core
projects q/k/v for its head group from its batch's activations, runs the
full S x S attention for its 4 heads, and writes ctx in [head, D, S] layout.
The host transposes/concatenates shards (not part of HW exec time).

Device kernel (per core, identical SPMD program, no collectives):
  - qT/kT computed directly in [D, S] layout (head pairs packed into 128
    partitions) so the scores matmul needs no transposes.
  - scoresT tiles [S_k=128, S_q] = kT_chunk.T @ qTz; softmax denominator via
    ones columns appended to v (one matmul stream produces ctx and denom).
  - exp on the scalar engine with the 1/sqrt(D) scale folded in; bf16
    matmul operands, f32 PSUM accumulation; final normalize = DVE divide.
  - padding mask folded into v_aug row zeroing (exp(x-1e4) underflows to 0
    in f32, so zeroing masked key rows is exactly equivalent).

v2 scheduling (vs the first working version):
  - 8 blocks B(p,qc); block 1 carries only its mandatory projections
    (kT0 all keys, qT0 qc0, all v chunks, plus Q01 prefetch); the
    remaining 10 projection groups are spread one or two per block so the
    scalar-engine-bound pair-1 phase keeps the PE busy too.
  - DMA: criticality-ordered big pieces; issue split across the SP and
    Scalar sequencers (each dma_start costs ~0.6us of sequencer time).
  - memset-fed warmup matmuls bridge the DMA head so the PE never idles
    long enough for the HAM clock governor to re-throttle to 1.2GHz.
  - epilogue per head: copy denominator psum->sbuf, fast reciprocal,
    multiply straight out of psum (no [128,512] staging copy).
"""

import numpy as np
import ml_dtypes

import concourse.bass as bass
import concourse.tile as tile
from concourse import bacc, mybir
from concourse.bass_utils import run_bass_kernel_spmd

B, S, H, D = 2, 2048, 16, 64
HID = H * D
NCORES = 8
HPC = 4               # heads per core
COLS = HPC * D        # 256 projection columns per core
KC = HID // 128       # 8 contraction chunks for projections
QC = S // 512         # 4 query chunks of 512
MC = S // 128         # 16 key chunks of 128

BF16 = mybir.dt.bfloat16
F32 = mybir.dt.float32
np_bf16 = ml_dtypes.bfloat16

N_WARM = 8            # memset-fed filler matmuls bridging the DMA head

_CACHE = {}


def build(apply_mask: bool) -> bass.Bass:
    nc = bacc.Bacc(None, target_bir_lowering=False, debug=False)

    xT = nc.declare_dram_parameter("xT", [HID, S], BF16, isOutput=False)
    wq = nc.declare_dram_parameter("wq", [HID, COLS], BF16, isOutput=False)
    wk = nc.declare_dram_parameter("wk", [HID, COLS], BF16, isOutput=False)
    wv = nc.declare_dram_parameter("wv", [HID, COLS], BF16, isOutput=False)
    bq = nc.declare_dram_parameter("bq", [128, 2], F32, isOutput=False)
    bk = nc.declare_dram_parameter("bk", [128, 2], F32, isOutput=False)
    bv = nc.declare_dram_parameter("bv", [128, COLS], F32, isOutput=False)
    if apply_mask:
        mm_in = nc.declare_dram_parameter("maskm", [128, MC], F32, isOutput=False)
    out_ext = nc.declare_dram_parameter("out", [HPC, D, S], F32, isOutput=True)

    with tile.TileContext(nc) as tc:
        with (
            tc.tile_pool(name="singles", bufs=1) as singles,
            tc.tile_pool(name="work", bufs=4) as work,
            tc.tile_pool(name="psum", bufs=2, space="PSUM") as psum,
        ):
            wq_sb = singles.tile([128, KC, COLS], BF16)
            wk_sb = singles.tile([128, KC, COLS], BF16)
            wv_sb = singles.tile([128, KC, COLS], BF16)
            x_sb = singles.tile([128, KC, S], BF16)
            bq_sb = singles.tile([128, 2], F32)
            bk_sb = singles.tile([128, 2], F32)
            bv_sb = singles.tile([128, COLS], F32)
            if apply_mask:
                mm_sb = singles.tile([128, MC], F32)
            zt = singles.tile([128, 512], BF16)

            # ---- input DMA ----
            # SP sequencer: the critical chain (pair-0 weights, then x in
            # consumption order).  Each dma_start costs ~565ns of sequencer
            # time, so pieces are big; each lands on its own DMA queue.
            def w_piece(eng, dst, src, pcols, j, nk):
                # nk contraction chunks of a [HID, COLS] weight, col slice
                eng.dma_start(
                    out=dst[:, j * nk:(j + 1) * nk, pcols],
                    in_=src[j * nk * 128:(j + 1) * nk * 128, pcols]
                        .rearrange("(k p) c -> p k c", p=128))

            def x_piece(eng, kc, qc):
                qsl = slice(qc * 512, (qc + 1) * 512)
                eng.dma_start(out=x_sb[:, kc, qsl],
                              in_=xT[kc * 128:(kc + 1) * 128, qsl])

            def x_half(eng, kc, half):
                csl = slice(half * 256, (half + 1) * 256)
                eng.dma_start(out=x_sb[:, kc, csl],
                              in_=xT[kc * 128:(kc + 1) * 128, csl])

            # x qc0 is the arrival-paced critical chain feeding K00/Q00:
            # 16 half-pieces spread over the SP, Scalar, and Vector
            # sequencers (each issue costs ~0.6us of sequencer time and a
            # 64KB piece transfers in ~1.5us on its own queue).
            w_piece(nc.sync, wk_sb, wk, slice(0, 128), 0, 4)
            x_half(nc.sync, 0, 0)
            w_piece(nc.sync, wk_sb, wk, slice(0, 128), 1, 4)
            for kc, half in ((1, 1), (3, 0), (4, 1), (6, 0), (7, 1)):
                x_half(nc.sync, kc, half)
            for j in range(2):
                w_piece(nc.sync, wq_sb, wq, slice(0, 128), j, 4)
            for qc in range(1, 3):
                for kc in range(KC):
                    x_piece(nc.sync, kc, qc)
            for j in range(4):  # qc3 as 4 double-kc pieces
                nc.sync.dma_start(
                    out=x_sb[:, 2 * j:2 * j + 2, 1536:2048],
                    in_=xT[j * 256:(j + 1) * 256, 1536:2048]
                        .rearrange("(k p) s -> p k s", p=128))

            # Scalar sequencer: more x qc0 pieces, then everything off the
            # critical chain (biases, wv, pair-1 weights).  Scalar is idle
            # until the first exp, after these issues drain.
            for kc, half in ((0, 1), (2, 0), (3, 1), (5, 0), (6, 1)):
                x_half(nc.scalar, kc, half)
            nc.scalar.dma_start(out=bk_sb, in_=bk[:, :])
            nc.scalar.dma_start(out=bq_sb, in_=bq[:, :])
            for j in range(4):
                w_piece(nc.scalar, wv_sb, wv, slice(0, COLS), j, 2)
            nc.scalar.dma_start(out=bv_sb, in_=bv[:, :])
            if apply_mask:
                nc.scalar.dma_start(out=mm_sb, in_=mm_in[:, :])
            for j in range(2):
                w_piece(nc.scalar, wk_sb, wk, slice(128, 256), j, 4)
            for j in range(2):
                w_piece(nc.scalar, wq_sb, wq, slice(128, 256), j, 4)

            # ---- static SBUF prep (DVE is idle during the DMA head) ----
            nc.vector.memset(zt, 0.0)
            # GpSimd sequencer issues the remaining x qc0 pieces (DVE can't
            # initiate DMAs; gpsimd is otherwise idle)
            for kc, half in ((1, 0), (2, 1), (4, 0), (5, 1), (7, 0)):
                x_half(nc.gpsimd, kc, half)
            # qTz zero halves are written once for the full S up front
            # (variant 0: head-b rows zero; variant 1: head-a rows zero)
            qTz = singles.tile([128, 2, 2, S], BF16)
            kT = singles.tile([128, 2, S], BF16)
            for p in range(2):
                nc.vector.memset(qTz[64:128, p, 0, :], 0.0)
                nc.vector.memset(qTz[0:64, p, 1, :], 0.0)
            # v_aug: [128, key_chunk, head, 128]; cols 64:128 are ones, so
            # the ctx matmul emits the softmax denominator into psum
            # partitions 64:128 at no extra cost (matmul cost is N-bound)
            v_aug = singles.tile([128, MC, HPC, 128], BF16)
            nc.vector.memset(v_aug[:, :, :, 64:128], 1.0)

            # ---- projection helpers ----
            def project_T_qc(dst, w_sb, b_sb, p, qc, zpad=False):
                ps = psum.tile([128, 512], F32, tag="proj_ps", name=f"pt_{nc.next_id()}")
                for kc in range(KC):
                    nc.tensor.matmul(
                        ps,
                        lhsT=w_sb[:, kc, p * 128:(p + 1) * 128],
                        rhs=x_sb[:, kc, qc * 512:(qc + 1) * 512],
                        start=(kc == 0), stop=(kc == KC - 1),
                    )
                qsl = slice(qc * 512, (qc + 1) * 512)
                if zpad:
                    nc.vector.tensor_tensor(
                        out=dst[0:64, p, 0, qsl],
                        in0=ps[0:64, :],
                        in1=b_sb[0:64, p:p + 1].to_broadcast([64, 512]),
                        op=mybir.AluOpType.add,
                    )
                    nc.vector.tensor_tensor(
                        out=dst[64:128, p, 1, qsl],
                        in0=ps[64:128, :],
                        in1=b_sb[64:128, p:p + 1].to_broadcast([64, 512]),
                        op=mybir.AluOpType.add,
                    )
                else:
                    nc.vector.tensor_tensor(
                        out=dst[:, p, qsl],
                        in0=ps,
                        in1=b_sb[:, p:p + 1].to_broadcast([128, 512]),
                        op=mybir.AluOpType.add,
                    )

            def project_v_chunk(mc):
                ps = psum.tile([128, COLS], F32, tag="proj_ps", name=f"pv_{nc.next_id()}")
                for kc in range(KC):
                    nc.tensor.matmul(
                        ps,
                        lhsT=x_sb[:, kc, mc * 128:(mc + 1) * 128],
                        rhs=wv_sb[:, kc, :],
                        start=(kc == 0), stop=(kc == KC - 1),
                    )
                nc.vector.tensor_tensor(
                    out=v_aug[:, mc, :, 0:64],
                    in0=ps[:, :].rearrange("p (h d) -> p h d", h=HPC),
                    in1=bv_sb.rearrange("p (h d) -> p h d", h=HPC),
                    op=mybir.AluOpType.add,
                )
                if apply_mask:
                    nc.vector.tensor_tensor(
                        out=v_aug[:, mc, :, :],
                        in0=v_aug[:, mc, :, :],
                        in1=mm_sb[:, mc:mc + 1, None].to_broadcast([128, HPC, 128]),
                        op=mybir.AluOpType.mult,
                    )

            # Named projection groups: K{p}{j} = kT pair p keys j*512..,
            # Q{p}{j} = qTz pair p queries j*512.. (zero-padded variants).
            def K(p, j):
                return lambda: project_T_qc(kT, wk_sb, bk_sb, p, j)

            def Q(p, j):
                return lambda: project_T_qc(qTz, wq_sb, bq_sb, p, j, zpad=True)

            def V2(mc):
                return lambda: (project_v_chunk(mc), project_v_chunk(mc + 1))

            def emit_ctx(pend):
                p, qc, kc2, e_a, e_b, ctx_a, ctx_b = pend
                ha, hb = 2 * p, 2 * p + 1
                for i, kc in enumerate((2 * kc2, 2 * kc2 + 1)):
                    csl = slice(i * 512, (i + 1) * 512)
                    nc.tensor.matmul(
                        ctx_a, lhsT=v_aug[:, kc, ha, :], rhs=e_a[:, csl],
                        start=(kc == 0), stop=(kc == MC - 1))
                    nc.tensor.matmul(
                        ctx_b, lhsT=v_aug[:, kc, hb, :], rhs=e_b[:, csl],
                        start=(kc == 0), stop=(kc == MC - 1))

            def emit_epilogue(p, qc, ctx_a, ctx_b, last=False):
                # denominator (psum rows 64:128) -> sbuf base-0 tile for the
                # custom-DVE reciprocal; multiply reads ctx psum directly so
                # there is no staging copy of the whole tile.  In the last
                # block the copy runs on the Scalar engine (idle after the
                # final exp) and the out-DMA issues are split across the SP
                # and Scalar sequencers to shorten the tail.
                for h, ctx in ((2 * p, ctx_a), (2 * p + 1, ctx_b)):
                    d0_sb = work.tile([64, 512], F32, tag="den0", name=f"d0_{nc.next_id()}")
                    nc.vector.tensor_copy(out=d0_sb, in_=ctx[64:128, :])
                    d_sb = work.tile([64, 512], F32, tag="den", name=f"d_{nc.next_id()}")
                    nc.vector.reciprocal_approx_fast(out=d_sb, in_=d0_sb)
                    o_sb = work.tile([64, 512], F32, tag="outt", name=f"o_{nc.next_id()}")
                    nc.vector.tensor_tensor(
                        out=o_sb, in0=ctx[0:64, :],
                        in1=d_sb,
                        op=mybir.AluOpType.mult)
                    for piece in range(2):
                        eng = (nc.scalar if piece else nc.sync) if last else nc.sync
                        eng.dma_start(
                            out=out_ext[h][:, qc * 512 + piece * 256:
                                           qc * 512 + (piece + 1) * 256],
                            in_=o_sb[:, piece * 256:(piece + 1) * 256])

            # ---- block schedule ----
            # B1 must own kT0 (all keys), qT0 qc0, every v chunk, and the
            # Q01 prefetch; the other 10 groups go one or two per block so
            # blocks 6-8 (scalar-bound: 16 exps = 16.6us vs 13.7us of
            # attention matmuls) still keep the PE fed.
            blocks = [(0, 0), (0, 1), (0, 2), (0, 3),
                      (1, 0), (1, 1), (1, 2), (1, 3)]
            hooks = [
                {0: V2(2), 1: lambda: (K(0, 1)(), V2(4)()),
                 2: V2(6), 3: lambda: (K(0, 2)(), V2(8)()),
                 4: V2(10), 5: lambda: (K(0, 3)(), V2(12)()),
                 6: V2(14), 7: Q(0, 1)},
                {1: Q(0, 2), 5: K(1, 0)},
                {1: Q(0, 3), 5: K(1, 1)},
                {1: Q(1, 0), 5: K(1, 2)},
                {1: Q(1, 1), 5: K(1, 3)},
                {1: Q(1, 2)},
                {1: Q(1, 3)},
                {},
            ]

            # ---- warmup + first projections ----
            # Filler matmuls on a zeroed tile bridge the DMA head (PE can
            # start at ~7us; first weight/x pieces land at ~13-15us), and
            # more fillers are interleaved into the DMA-arrival-paced K00 /
            # Q00 chains so the PE never idles long enough for the HAM
            # clock governor to drop back to 1.2GHz.
            warm_ps = psum.tile([128, 512], F32, tag="proj_ps", name="warm_ps")
            k00_ps = psum.tile([128, 512], F32, tag="proj_ps", name="k00_ps")

            def warm(n, cols=512):
                # singleton-group filler matmuls (start+stop, no accumulate)
                for _ in range(n):
                    nc.tensor.matmul(warm_ps[:, 0:cols], lhsT=zt[:, 0:128],
                                     rhs=zt[:, 0:cols], start=True, stop=True)

            warm(N_WARM)
            # K00 is paced by the x qc0 piece arrivals (~0.5-1us apart);
            # short N=128 fillers between steps keep the PE busy so the
            # clock ramp (needs ~3us continuous) isn't reset by the waits.
            for kc in range(KC):
                nc.tensor.matmul(
                    k00_ps, lhsT=wk_sb[:, kc, 0:128],
                    rhs=x_sb[:, kc, 0:512],
                    start=(kc == 0), stop=(kc == KC - 1))
                if kc < KC - 1:
                    warm(5, cols=128)
            nc.vector.tensor_tensor(
                out=kT[:, 0, 0:512], in0=k00_ps,
                in1=bk_sb[:, 0:1].to_broadcast([128, 512]),
                op=mybir.AluOpType.add)
            project_T_qc(qTz, wq_sb, bq_sb, 0, 0, True)   # Q00
            project_v_chunk(0)
            project_v_chunk(1)

            # ---- software-pipelined attention ----
            # ctx matmuls lag one (block, kc2) step behind the score
            # matmuls, across block boundaries, so the PE computes the next
            # scores while the scalar engine is still on the previous exps
            # (the scalar engine is the per-block bottleneck for pair 1).
            pend = None
            for bi, (p, qc) in enumerate(blocks):
                qsl = slice(qc * 512, (qc + 1) * 512)
                ctx_a = psum.tile([128, 512], F32, tag="ctx", name=f"ca_{nc.next_id()}")
                ctx_b = psum.tile([128, 512], F32, tag="ctx", name=f"cb_{nc.next_id()}")
                for kc2 in range(MC // 2):
                    kc0, kc1 = 2 * kc2, 2 * kc2 + 1
                    s_a = psum.tile([128, 1024], F32, tag="sps", name=f"sa_{nc.next_id()}")
                    s_b = psum.tile([128, 1024], F32, tag="sps", name=f"sb_{nc.next_id()}")
                    # paired row-group matmuls: head a on array rows 0:63,
                    # head b on rows 64:127 run concurrently
                    for i, kc in enumerate((kc0, kc1)):
                        ksl = slice(kc * 128, (kc + 1) * 128)
                        csl = slice(i * 512, (i + 1) * 512)
                        nc.tensor.matmul(
                            s_a[:, csl], lhsT=kT[:, p, ksl], rhs=qTz[:, p, 0, qsl],
                            start=True, stop=True)
                        nc.tensor.matmul(
                            s_b[:, csl], lhsT=kT[:, p, ksl], rhs=qTz[:, p, 1, qsl],
                            start=True, stop=True)
                    hook = hooks[bi].get(kc2)
                    if hook is not None:
                        hook()
                    e_a = work.tile([128, 1024], BF16, tag="expT", name=f"ea_{nc.next_id()}")
                    e_b = work.tile([128, 1024], BF16, tag="expT", name=f"eb_{nc.next_id()}")
                    nc.scalar.activation(e_a, s_a, mybir.ActivationFunctionType.Exp,
                                         scale=0.125)
                    nc.scalar.activation(e_b, s_b, mybir.ActivationFunctionType.Exp,
                                         scale=0.125)
                    if pend is not None:
                        emit_ctx(pend)
                        if pend[2] == MC // 2 - 1:
                            emit_epilogue(pend[0], pend[1], pend[5], pend[6])
                    pend = (p, qc, kc2, e_a, e_b, ctx_a, ctx_b)
            emit_ctx(pend)
            emit_epilogue(pend[0], pend[1], pend[5], pend[6], last=True)

    nc.compile()
    return nc


def _get_nc(apply_mask: bool) -> bass.Bass:
    if apply_mask not in _CACHE:
        _CACHE[apply_mask] = build(apply_mask)
    return _CACHE[apply_mask]


def _in_maps(x, mask, Wq, bq, Wk, bk, Wv, bv, apply_mask):
    xT_b = [np.ascontiguousarray(x[b].T).astype(np_bf16) for b in range(B)]
    maps = []
    for c in range(NCORES):
        b, hg = c // 4, c % 4
        cs = slice(hg * COLS, (hg + 1) * COLS)
        m = {
            "xT": xT_b[b],
            "wq": np.ascontiguousarray(Wq[:, cs]).astype(np_bf16),
            "wk": np.ascontiguousarray(Wk[:, cs]).astype(np_bf16),
            "wv": np.ascontiguousarray(Wv[:, cs]).astype(np_bf16),
            "bq": np.ascontiguousarray(bq[cs].reshape(2, 128).T).astype(np.float32),
            "bk": np.ascontiguousarray(bk[cs].reshape(2, 128).T).astype(np.float32),
            "bv": np.ascontiguousarray(
                np.broadcast_to(bv[cs], (128, COLS))).astype(np.float32),
        }
        if apply_mask:
            m["maskm"] = np.ascontiguousarray(
                mask[b].astype(np.float32).reshape(MC, 128).T)
        maps.append(m)
    return maps


def _ensure_ntff_hook():
    """The agent image's antenv lacks axon_hooks; synthesize it so
    run_bass_kernel_spmd(trace=True) can reach the axon NTFF profiler."""
    import sys as _sys
    import types as _types
    try:
        from antenv import axon_hooks  # noqa: F401
        return
    except ImportError:
        pass
    import antenv
    mod = _types.ModuleType("antenv.axon_hooks")
    _hook = [None]
    mod.set_axon_ntff_profile_hook = lambda h: _hook.__setitem__(0, h)
    mod.get_axon_ntff_profile_hook = lambda: _hook[0]
    _sys.modules["antenv.axon_hooks"] = mod
    antenv.axon_hooks = mod
    from trn_agent_boot.trn_boot import _ntff_profile_via_ctypes
    mod.set_axon_ntff_profile_hook(
        _ntff_profile_via_ctypes("/opt/axon/libaxon_pjrt.so"))


def run(inputs: dict, trace: bool = False):
    if trace:
        _ensure_ntff_hook()
    x = np.asarray(inputs["x"], dtype=np.float32)
    mask = np.asarray(inputs["mask"])
    apply_mask = not bool((mask == 1).all())
    nc = _get_nc(apply_mask)
    maps = _in_maps(x, mask, np.asarray(inputs["Wq"], np.float32),
                    np.asarray(inputs["bq"], np.float32),
                    np.asarray(inputs["Wk"], np.float32),
                    np.asarray(inputs["bk"], np.float32),
                    np.asarray(inputs["Wv"], np.float32),
                    np.asarray(inputs["bv"], np.float32), apply_mask)
    res = run_bass_kernel_spmd(nc, maps, core_ids=list(range(NCORES)), trace=trace)
    out = np.empty((B, S, HID), dtype=np.float32)
    for c in range(NCORES):
        b, hg = c // 4, c % 4
        cs = slice(hg * COLS, (hg + 1) * COLS)
        ctxT = res.results[c]["out"]          # [HPC, D, S]
        out[b, :, cs] = ctxT.transpose(2, 0, 1).reshape(S, COLS)
    return out, res


def kernel(**inputs) -> np.ndarray:
    out, _ = run(inputs)
    return out
